# revision 6
# baseline (speedup 1.0000x reference)
"""Trainium2 Bass kernel for nn_BoundaryLoss: mean(|softmax(pred) * SDF(onehot(target))|).

Strategy (8 NeuronCores, SPMD, one (b, c) pair per core):
  - Exact 3D squared EDT of the class-c mask (pos) and complement (neg) via
    separable passes. |sdf| = sqrt(g_pos) + sqrt(g_neg) since exactly one of
    the two is zero at every voxel.
  - W pass: two tensor_tensor_scan ops (fwd chamfer, then bwd chamfer chained
    on the fwd result) give the exact 1D L1 seed distance per row; one Square
    activation turns it into the squared EDT. INF pads between rows stop the
    recurrence from leaking across (h) rows.
  - H pass: 6 scalar_tensor_tensor min-plus shift updates (s = 1..3, both
    directions, truncation certified exact on the host). First op fused with
    the pass-init copy.
  - D pass: same, with the partition-axis shifts materialized by SBUF->SBUF
    DMA copies (borders filled from an INF tile; compute ops never straddle
    non-aligned partition starts).
  - Softmax: pred DMA'd in layout [h2*64+d, (c, h%24, w)] with class c
    permuted to slot 0; exp on the scalar engine; 4-way denominator adds,
    reciprocal_approx_fast and the weight multiply on DVE; two fused
    tensor_tensor_reduce ops against the PE pair-sum produce 48 partials.
  - Host sums the 8x48 partials, applies the has_pos gate and the mean factor.

Engine budget: DVE ~42us is the critical path; Act (init/exp/sqrt), pool
(neg-init, pad memsets), PE (pair-sum matmul) and the DMA queue all hide
under it.
"""

import os
import sys

import numpy as np

B, C, DD, HH, WW = 2, 4, 48, 48, 48
PLANE = HH * WW  # 2304
WPAD = WW + 1  # 49: w row + INF pad column
NFP = HH * WPAD  # 2352 padded free size
NVOX = DD * PLANE
S_MAX = 16
S_DEV = 3  # device kernel is built for shift radius 3; larger -> fallback
N_CORES = 8
RV = 112  # end of valid rows: pos [0:48) | gap [48:64) | neg [64:112)
INF_I = 30000.0  # scan/pad infinity (int16 domain)
INF_SEED = 1024.0  # neg-mask seed infinity (= 32^2, matches Square scaling)

_nc_cache = {}
LAST_RESULTS = None  # test harness introspection


def _ensure_paths():
    for p in ("/opt/trn_rl_repo",):
        if os.path.isdir(p) and p not in sys.path:
            sys.path.insert(0, p)


def _edt_sq_trunc_np(f0, S):
    """Truncated-shift separable squared EDT (numpy, int32). Mirrors the device
    algorithm; used for the shift-bound certification and the fallback path."""
    f = f0.astype(np.int32)
    for ax in (2, 1, 0):
        g = f.copy()
        for s in range(1, S + 1):
            s2 = s * s
            sl_out = [slice(None)] * 3
            sl_in = [slice(None)] * 3
            sl_out[ax] = slice(s, None)
            sl_in[ax] = slice(None, -s)
            np.minimum(g[tuple(sl_out)], f[tuple(sl_in)] + s2, out=g[tuple(sl_out)])
            sl_out[ax] = slice(None, -s)
            sl_in[ax] = slice(s, None)
            np.minimum(g[tuple(sl_out)], f[tuple(sl_in)] + s2, out=g[tuple(sl_out)])
        f = g
    return f


def _certified_shift_bound(masks):
    """Smallest S such that the S-truncated separable EDT is provably exact for
    every seed mask in `masks` (max truncated distance <= S certifies that no
    winning chain was cut off). The device kernel's exact-W variant is then
    also exact, since relaxing one pass can only move the result toward the
    true EDT."""
    for S in range(1, S_MAX + 1):
        worst = 0
        for m in masks:
            f0 = np.where(m, 0, 30000).astype(np.int16)
            g = _edt_sq_trunc_np(f0, S)
            worst = max(worst, int(np.ceil(np.sqrt(float(g.max())))))
        if worst <= S:
            return S
    return S_MAX + 1


def _reference_fallback(pred, target):
    """Exact numpy replica of the reference for pathological inputs the device
    path does not cover (wrong shapes, class filling a volume, S > S_DEV)."""
    INF = 1e9
    pred = np.asarray(pred, np.float32)
    target = np.asarray(target)
    b_, c_ = pred.shape[0], pred.shape[1]
    n = np.arange(pred.shape[-1])

    def minplus(f):
        d2 = ((n[:, None] - n[None, :]) ** 2).astype(np.float32)
        return (f[..., None, :] + d2).min(axis=-1)

    def edt(src):
        f = np.where(src, 0.0, INF).astype(np.float32)
        for ax in (-3, -2, -1):
            f = np.moveaxis(minplus(np.moveaxis(f, ax, -1)), -1, ax)
        return np.sqrt(f)

    e = np.exp(pred - pred.max(axis=1, keepdims=True))
    sm = e / e.sum(axis=1, keepdims=True)
    total = 0.0
    for b in range(b_):
        for c in range(c_):
            pos = target[b] == c
            if not pos.any():
                continue
            sdf = edt(pos) - edt(~pos)
            total += float(np.abs(sm[b, c] * sdf).sum(dtype=np.float64))
    return np.float32(total / pred.size)


def _build_nc():
    _ensure_paths()
    import concourse.tile as tile
    from concourse import bacc, mybir

    i16 = mybir.dt.int16
    f32 = mybir.dt.float32
    ALU = mybir.AluOpType
    ACT = mybir.ActivationFunctionType

    NP = 128

    nc = bacc.Bacc("TRN2", target_bir_lowering=False, debug=False)

    tgt_d = nc.dram_tensor("tgt", [128, PLANE], i16, kind="ExternalInput")
    cvn_d = nc.dram_tensor("cvn", [NP, 1], f32, kind="ExternalInput")
    cvc_d = nc.dram_tensor("cvc", [NP, 1], f32, kind="ExternalInput")
    ones_d = nc.dram_tensor("ones", [NP, NFP], i16, kind="ExternalInput")
    inf_d = nc.dram_tensor("inf", [4, PLANE], f32, kind="ExternalInput")
    sm_d = nc.dram_tensor("predsm", [2, 48, 4 * 1152], f32, kind="ExternalInput")
    pm_d = nc.dram_tensor("pairmat", [NP, 48], f32, kind="ExternalInput")
    out_d = nc.dram_tensor("out", [48, 2], f32, kind="ExternalOutput")

    with tile.TileContext(nc) as tc:
        with (
            tc.tile_pool(name="main", bufs=1) as pool,
            tc.tile_pool(name="psum", bufs=1, space="PSUM") as psp,
        ):
            T64 = pool.tile([NP, PLANE], i16, tag="T64")
            CVN = pool.tile([NP, 1], f32, tag="CVN")
            CVC = pool.tile([NP, 1], f32, tag="CVC")
            ONES = pool.tile([NP, NFP], i16, tag="ONES")
            INF = pool.tile([4, PLANE], f32, tag="INF")
            SM = pool.tile([NP, 4 * 1152], f32, tag="SM")
            SME = pool.tile([NP, 4 * 1152], f32, tag="SME")
            PM = pool.tile([NP, 48], f32, tag="PM")
            F = pool.tile([NP, NFP], i16, tag="F")
            WF = pool.tile([NP, NFP], i16, tag="WF")
            WB = pool.tile([NP, NFP], i16, tag="WB")
            A = pool.tile([NP, PLANE], f32, tag="A")
            Bt = pool.tile([NP, PLANE], f32, tag="B")
            SQ = pool.tile([NP, PLANE], f32, tag="SQ")
            DN = pool.tile([NP, 1152], f32, tag="DN")
            RC = pool.tile([NP, 1152], f32, tag="RC")
            WT = pool.tile([NP, 1152], f32, tag="WT")
            SCR = pool.tile([48, 1152], f32, tag="SCR")
            SCR2 = pool.tile([48, 1152], f32, tag="SCR2")
            ACO = pool.tile([48, 2], f32, tag="ACO")
            FS = []
            for j in range(6):
                fsj = pool.tile([NP, PLANE], f32, tag=f"fs{j}", name=f"fs{j}")
                FS.append(fsj)
            PS = psp.tile([48, PLANE], f32, tag="ps")

            # ---- input DMAs -------------------------------------------------
            nc.sync.dma_start(T64[:], tgt_d[:])
            nc.sync.dma_start(CVN[:], cvn_d[:])
            nc.sync.dma_start(CVC[:], cvc_d[:])
            nc.sync.dma_start(ONES[:], ones_d[:])
            nc.sync.dma_start(INF[:], inf_d[:])
            nc.sync.dma_start(SM[0:48, :], sm_d[0])
            nc.sync.dma_start(SM[64:RV, :], sm_d[1])
            nc.sync.dma_start(PM[:], pm_d[:])

            Fp = F[:].rearrange("p (h w) -> p h w", w=WPAD)

            # pool: pad columns of F to INF, SQ tail to 0, neg-mask init
            nc.gpsimd.memset(Fp[0:RV, :, WW : WW + 1], INF_I)
            nc.gpsimd.memset(SQ[96:NP, :], 0.0)  # sqrt later rewrites [96:RV)
            nc.gpsimd.tensor_scalar(
                out=Fp[64:RV, :, 0:WW], in0=T64[64:RV, :].rearrange(
                    "p (h w) -> p h w", w=WW
                ), scalar1=CVC[64:RV, :], scalar2=INF_SEED,
                op0=ALU.is_equal, op1=ALU.mult,
            )

            # Act: pos-mask init f = (32*(t-c))^2 over pos+gap rows (gap rows
            # carry the t=4 sentinel -> >= INF_SEED, isolating the blocks)
            nc.scalar.activation(
                Fp[0:64, :, 0:WW],
                T64[0:64, :].rearrange("p (h w) -> p h w", w=WW),
                ACT.Square, bias=CVN[0:64, :], scale=32.0,
            )

            # ---- W pass: fwd scan, bwd scan chained on fwd, then square -----
            nc.vector.tensor_tensor_scan(
                out=WF[0:RV, :], data0=ONES[0:RV, :], data1=F[0:RV, :],
                initial=INF_I, op0=ALU.add, op1=ALU.min,
            )
            nc.vector.tensor_tensor_scan(
                out=WB[0:RV, ::-1], data0=ONES[0:RV, ::-1], data1=WF[0:RV, ::-1],
                initial=INF_I, op0=ALU.add, op1=ALU.min,
            )
            A3 = A[:].rearrange("p (h w) -> p h w", w=WW)
            B3 = Bt[:].rearrange("p (h w) -> p h w", w=WW)
            nc.scalar.activation(
                A3[0:RV, :, :],
                WB[:].rearrange("p (h w) -> p h w", w=WPAD)[0:RV, :, 0:WW],
                ACT.Square,
            )

            # ---- H pass: A -> Bt, 6 truncated min-plus shifts ---------------
            nc.vector.scalar_tensor_tensor(
                out=B3[0:RV, 1:HH, :], in0=A3[0:RV, 0 : HH - 1, :], scalar=1.0,
                in1=A3[0:RV, 1:HH, :], op0=ALU.add, op1=ALU.min,
            )
            nc.vector.tensor_copy(B3[0:RV, 0:1, :], A3[0:RV, 0:1, :])
            nc.vector.scalar_tensor_tensor(
                out=B3[0:RV, 0 : HH - 1, :], in0=A3[0:RV, 1:HH, :], scalar=1.0,
                in1=B3[0:RV, 0 : HH - 1, :], op0=ALU.add, op1=ALU.min,
            )
            for s in (2, 3):
                s2 = float(s * s)
                nc.vector.scalar_tensor_tensor(
                    out=B3[0:RV, s:HH, :], in0=A3[0:RV, 0 : HH - s, :],
                    scalar=s2, in1=B3[0:RV, s:HH, :], op0=ALU.add, op1=ALU.min,
                )
                nc.vector.scalar_tensor_tensor(
                    out=B3[0:RV, 0 : HH - s, :], in0=A3[0:RV, s:HH, :],
                    scalar=s2, in1=B3[0:RV, 0 : HH - s, :],
                    op0=ALU.add, op1=ALU.min,
                )

            # ---- D pass: Bt -> A via partition-shifted DMA copies -----------
            shifts = [(1, 1), (1, -1), (2, 1), (2, -1), (3, 1), (3, -1)]
            for j, (s, sign) in enumerate(shifts):
                fs = FS[j]
                if sign > 0:
                    nc.sync.dma_start(fs[0:s, :], INF[0:s, :])
                    nc.sync.dma_start(fs[s:RV, :], Bt[0 : RV - s, :])
                else:
                    nc.sync.dma_start(fs[0 : RV - s, :], Bt[s:RV, :])
                    nc.sync.dma_start(fs[RV - s : RV, :], INF[0:s, :])
            nc.vector.scalar_tensor_tensor(
                out=A[0:RV, :], in0=FS[0][0:RV, :], scalar=1.0,
                in1=Bt[0:RV, :], op0=ALU.add, op1=ALU.min,
            )
            for j, (s, sign) in enumerate(shifts):
                if j == 0:
                    continue
                nc.vector.scalar_tensor_tensor(
                    out=A[0:RV, :], in0=FS[j][0:RV, :], scalar=float(s * s),
                    in1=A[0:RV, :], op0=ALU.add, op1=ALU.min,
                )

            # ---- |sdf| = sqrt(g_pos) + sqrt(g_neg) --------------------------
            nc.scalar.activation(SQ[0:RV, :], A[0:RV, :], ACT.Sqrt)
            n0 = 0
            while n0 < PLANE:
                nn = min(512, PLANE - n0)
                nc.tensor.matmul(
                    PS[:, n0 : n0 + nn], PM[:], SQ[:, n0 : n0 + nn],
                    start=True, stop=True,
                )
                n0 += nn

            # ---- softmax weight for class c (slot 0 after host permute) -----
            nc.scalar.activation(SME[0:RV, :], SM[0:RV, :], ACT.Exp)
            nc.vector.tensor_tensor(
                DN[0:RV, :], SME[0:RV, 0:1152], SME[0:RV, 1152:2304], ALU.add
            )
            nc.vector.tensor_tensor(
                DN[0:RV, :], DN[0:RV, :], SME[0:RV, 2304:3456], ALU.add
            )
            nc.vector.tensor_tensor(
                DN[0:RV, :], DN[0:RV, :], SME[0:RV, 3456:4608], ALU.add
            )
            nc.vector.reciprocal_approx_fast(out=RC[0:RV, :], in_=DN[0:RV, :])
            nc.vector.tensor_tensor(
                WT[0:RV, :], SME[0:RV, 0:1152], RC[0:RV, :], ALU.mult
            )

            # ---- partial[d] = sum |sdf| * w_c  (TT mults + Act accum) -------
            nc.vector.tensor_tensor(SCR[:], PS[:, 0:1152], WT[0:48, :], ALU.mult)
            nc.scalar.activation(SCR[:], SCR[:], ACT.Copy, accum_out=ACO[:, 0:1])
            nc.vector.tensor_tensor(
                SCR2[:], PS[:, 1152:2304], WT[64:RV, :], ALU.mult
            )
            nc.scalar.activation(SCR2[:], SCR2[:], ACT.Copy, accum_out=ACO[:, 1:2])
            nc.sync.dma_start(out_d[:], ACO[:])

    nc.compile()
    return nc


def kernel(pred, target):
    pred = np.ascontiguousarray(np.asarray(pred), dtype=np.float32)
    target = np.asarray(target)

    if pred.shape != (B, C, DD, HH, WW) or target.shape != (B, DD, HH, WW):
        return _reference_fallback(pred, target)

    tgt = target.astype(np.int64)
    masks = []
    has_pos = {}
    for b in range(B):
        for c in range(C):
            m = tgt[b] == c
            has_pos[(b, c)] = bool(m.any())
            if has_pos[(b, c)]:
                masks.append(m)
                mn = ~m
                if mn.any():
                    masks.append(mn)
                else:
                    return _reference_fallback(pred, target)  # class fills volume

    if _certified_shift_bound(masks) > S_DEV:
        return _reference_fallback(pred, target)

    _ensure_paths()
    from concourse.bass_utils import run_bass_kernel_spmd

    if 0 not in _nc_cache:
        _nc_cache[0] = _build_nc()
    nc = _nc_cache[0]

    NP = 128

    ones = np.ones((NP, NFP), np.int16)
    ones[:, WW::WPAD] = int(INF_I)
    inf_t = np.full((4, PLANE), 1e9, np.float32)
    pairmat = np.zeros((NP, 48), np.float32)
    pairmat[np.arange(48), np.arange(48)] = 1.0
    pairmat[64 + np.arange(48), np.arange(48)] = 1.0

    in_maps = []
    for k in range(N_CORES):
        b, c = divmod(k, C)
        t16 = tgt[b].reshape(DD, PLANE).astype(np.int16)
        T = np.empty((128, PLANE), np.int16)
        T[0:48] = t16
        T[48:64] = 4  # sentinel: (4-c)^2 * 1024 >= 1024 isolates the blocks
        T[64:112] = t16
        T[112:128] = 4
        cvn = np.full((NP, 1), -32.0 * c, np.float32)
        cvc = np.full((NP, 1), float(c), np.float32)
        perm = [c] + [j for j in range(C) if j != c]
        # [h2*64+d, (c', h%24, w)] layout with class c in slot 0
        sm = (
            pred[b][perm]
            .reshape(C, DD, 2, 24, WW)
            .transpose(2, 1, 0, 3, 4)
            .reshape(2, 48, 4 * 1152)
        )
        in_maps.append(
            {
                "tgt": T,
                "cvn": cvn,
                "cvc": cvc,
                "ones": ones,
                "inf": inf_t,
                "predsm": np.ascontiguousarray(sm),
                "pairmat": pairmat,
            }
        )

    trace = bool(os.environ.get("BOUNDARY_KERNEL_TRACE"))
    if trace:
        import importlib.util

        if importlib.util.find_spec("antenv.axon_hooks") is None:
            trace = False
    res = run_bass_kernel_spmd(nc, in_maps, list(range(N_CORES)), trace=trace)
    global LAST_RESULTS
    LAST_RESULTS = res

    total = 0.0
    for k in range(N_CORES):
        b, c = divmod(k, C)
        if has_pos[(b, c)]:
            total += float(res.results[k]["out"].astype(np.float64).sum())
    return np.float32(total / (B * C * NVOX))


if __name__ == "__main__":
    import reference

    inputs = reference.setup_inputs()
    out = kernel(**{k: np.asarray(v) for k, v in inputs.items()})
    print("kernel out:", out)


# revision 8
# speedup vs baseline: 1.0783x; 1.0783x over previous
"""Trainium2 Bass kernel for nn_BoundaryLoss: mean(|softmax(pred) * SDF(onehot(target))|).

Strategy (8 NeuronCores, SPMD, one (b, c) pair per core):
  - Exact 3D squared EDT of the class-c mask (pos) and complement (neg) via
    separable passes. |sdf| = sqrt(g_pos) + sqrt(g_neg) since exactly one of
    the two is zero at every voxel.
  - W pass: two tensor_tensor_scan ops (fwd chamfer, then bwd chamfer chained
    on the fwd result) give the exact 1D L1 seed distance per row; one Square
    activation turns it into the squared EDT. INF pads between rows stop the
    recurrence from leaking across (h) rows.
  - H pass: 6 scalar_tensor_tensor min-plus shift updates (s = 1..3, both
    directions, truncation certified exact on the host). First op fused with
    the pass-init copy.
  - D pass: same, with the partition-axis shifts materialized by SBUF->SBUF
    DMA copies (borders filled from an INF tile; compute ops never straddle
    non-aligned partition starts).
  - Softmax: pred DMA'd in layout [h2*64+d, (c, h%24, w)] with class c
    permuted to slot 0; exp on the scalar engine; 4-way denominator adds,
    reciprocal_approx_fast and the weight multiply on DVE; two fused
    tensor_tensor_reduce ops against the PE pair-sum produce 48 partials.
  - Host sums the 8x48 partials, applies the has_pos gate and the mean factor.

Engine budget: DVE ~42us is the critical path; Act (init/exp/sqrt), pool
(neg-init, pad memsets), PE (pair-sum matmul) and the DMA queue all hide
under it.
"""

import os
import sys

import numpy as np

B, C, DD, HH, WW = 2, 4, 48, 48, 48
PLANE = HH * WW  # 2304
WPAD = WW + 1  # 49: w row + INF pad column
NFP = HH * WPAD  # 2352 padded free size
NVOX = DD * PLANE
S_MAX = 16
S_DEV = 3  # device kernel is built for shift radius 3; larger -> fallback
N_CORES = 8
RV = 112  # end of valid rows: pos [0:48) | gap [48:64) | neg [64:112)
INF_I = 30000.0  # scan/pad infinity (int16 domain)
INF_SEED = 1024.0  # neg-mask seed infinity (= 32^2, matches Square scaling)

_nc_cache = {}
LAST_RESULTS = None  # test harness introspection


def _ensure_paths():
    for p in ("/opt/trn_rl_repo",):
        if os.path.isdir(p) and p not in sys.path:
            sys.path.insert(0, p)


def _edt_sq_trunc_np(f0, S):
    """Truncated-shift separable squared EDT (numpy, int32). Mirrors the device
    algorithm; used for the shift-bound certification and the fallback path."""
    f = f0.astype(np.int32)
    for ax in (2, 1, 0):
        g = f.copy()
        for s in range(1, S + 1):
            s2 = s * s
            sl_out = [slice(None)] * 3
            sl_in = [slice(None)] * 3
            sl_out[ax] = slice(s, None)
            sl_in[ax] = slice(None, -s)
            np.minimum(g[tuple(sl_out)], f[tuple(sl_in)] + s2, out=g[tuple(sl_out)])
            sl_out[ax] = slice(None, -s)
            sl_in[ax] = slice(s, None)
            np.minimum(g[tuple(sl_out)], f[tuple(sl_in)] + s2, out=g[tuple(sl_out)])
        f = g
    return f


def _certified_shift_bound(masks):
    """Smallest S such that the S-truncated separable EDT is provably exact for
    every seed mask in `masks` (max truncated distance <= S certifies that no
    winning chain was cut off). The device kernel's exact-W variant is then
    also exact, since relaxing one pass can only move the result toward the
    true EDT."""
    for S in range(1, S_MAX + 1):
        worst = 0
        for m in masks:
            f0 = np.where(m, 0, 30000).astype(np.int16)
            g = _edt_sq_trunc_np(f0, S)
            worst = max(worst, int(np.ceil(np.sqrt(float(g.max())))))
        if worst <= S:
            return S
    return S_MAX + 1


def _reference_fallback(pred, target):
    """Exact numpy replica of the reference for pathological inputs the device
    path does not cover (wrong shapes, class filling a volume, S > S_DEV)."""
    INF = 1e9
    pred = np.asarray(pred, np.float32)
    target = np.asarray(target)
    b_, c_ = pred.shape[0], pred.shape[1]
    n = np.arange(pred.shape[-1])

    def minplus(f):
        d2 = ((n[:, None] - n[None, :]) ** 2).astype(np.float32)
        return (f[..., None, :] + d2).min(axis=-1)

    def edt(src):
        f = np.where(src, 0.0, INF).astype(np.float32)
        for ax in (-3, -2, -1):
            f = np.moveaxis(minplus(np.moveaxis(f, ax, -1)), -1, ax)
        return np.sqrt(f)

    e = np.exp(pred - pred.max(axis=1, keepdims=True))
    sm = e / e.sum(axis=1, keepdims=True)
    total = 0.0
    for b in range(b_):
        for c in range(c_):
            pos = target[b] == c
            if not pos.any():
                continue
            sdf = edt(pos) - edt(~pos)
            total += float(np.abs(sm[b, c] * sdf).sum(dtype=np.float64))
    return np.float32(total / pred.size)


def _build_nc():
    _ensure_paths()
    import concourse.tile as tile
    from concourse import bacc, mybir

    i16 = mybir.dt.int16
    f32 = mybir.dt.float32
    ALU = mybir.AluOpType
    ACT = mybir.ActivationFunctionType

    NP = 128

    nc = bacc.Bacc("TRN2", target_bir_lowering=False, debug=False)

    tgt_d = nc.dram_tensor("tgt", [128, PLANE], i16, kind="ExternalInput")
    cvn_d = nc.dram_tensor("cvn", [NP, 1], f32, kind="ExternalInput")
    cvc_d = nc.dram_tensor("cvc", [NP, 1], f32, kind="ExternalInput")
    ones_d = nc.dram_tensor("ones", [NP, NFP], i16, kind="ExternalInput")
    sm_d = nc.dram_tensor("predsm", [2, 48, 4 * 1152], f32, kind="ExternalInput")
    pm_d = nc.dram_tensor("pairmat", [NP, 48], f32, kind="ExternalInput")
    out_d = nc.dram_tensor("out", [48, 2], f32, kind="ExternalOutput")

    with tile.TileContext(nc) as tc:
        with (
            tc.tile_pool(name="main", bufs=1) as pool,
            tc.tile_pool(name="psum", bufs=1, space="PSUM") as psp,
        ):
            T64 = pool.tile([NP, PLANE], i16, tag="T64")
            CVN = pool.tile([NP, 1], f32, tag="CVN")
            CVC = pool.tile([NP, 1], f32, tag="CVC")
            ONES = pool.tile([NP, NFP], i16, tag="ONES")
            SM = pool.tile([NP, 4 * 1152], f32, tag="SM")
            SME = pool.tile([NP, 4 * 1152], f32, tag="SME")
            PM = pool.tile([NP, 48], f32, tag="PM")
            F = pool.tile([NP, NFP], i16, tag="F")
            WF = pool.tile([NP, NFP], i16, tag="WF")
            WB = pool.tile([NP, NFP], i16, tag="WB")
            A = pool.tile([NP, PLANE], f32, tag="A")
            Bt = pool.tile([NP, PLANE], f32, tag="B")
            SQ = pool.tile([NP, PLANE], f32, tag="SQ")
            DN = pool.tile([NP, 1152], f32, tag="DN")
            RC = pool.tile([NP, 1152], f32, tag="RC")
            WT = pool.tile([NP, 1152], f32, tag="WT")
            SCR = pool.tile([48, 1152], f32, tag="SCR")
            SCR2 = pool.tile([48, 1152], f32, tag="SCR2")
            ACO = pool.tile([48, 2], f32, tag="ACO")
            FS = []
            for j in range(6):
                fsj = pool.tile([NP, PLANE], f32, tag=f"fs{j}", name=f"fs{j}")
                FS.append(fsj)
            PS = psp.tile([48, PLANE], f32, tag="ps")

            # ---- input DMAs -------------------------------------------------
            nc.sync.dma_start(T64[:], tgt_d[:])
            nc.sync.dma_start(CVN[:], cvn_d[:])
            nc.sync.dma_start(CVC[:], cvc_d[:])
            nc.sync.dma_start(ONES[:], ones_d[:])
            nc.sync.dma_start(SM[0:48, :], sm_d[0])
            nc.sync.dma_start(SM[64:RV, :], sm_d[1])
            nc.sync.dma_start(PM[:], pm_d[:])

            Fp = F[:].rearrange("p (h w) -> p h w", w=WPAD)

            # pool: pad columns of F to INF, SQ tail to 0, neg-mask init
            nc.gpsimd.memset(Fp[0:RV, :, WW : WW + 1], INF_I)
            nc.gpsimd.memset(SQ[96:NP, :], 0.0)  # sqrt later rewrites [96:RV)
            for j, (s, sign) in enumerate(
                [(1, 1), (1, -1), (2, 1), (2, -1), (3, 1), (3, -1)]
            ):
                if sign > 0:
                    nc.gpsimd.memset(FS[j][0:32, :], 1e9)
                else:
                    nc.gpsimd.memset(FS[j][96:RV, :], 1e9)

            # Act: pos-mask init f = (32*(t-c))^2 over pos+gap rows (gap rows
            # carry the t=4 sentinel -> >= INF_SEED, isolating the blocks)
            nc.scalar.activation(
                Fp[0:64, :, 0:WW],
                T64[0:64, :].rearrange("p (h w) -> p h w", w=WW),
                ACT.Square, bias=CVN[0:64, :], scale=32.0,
            )
            nc.scalar.activation(
                Fp[64:RV, :, 0:WW], Fp[0:48, :, 0:WW],
                ACT.Relu, bias=CVC[64:RV, :], scale=-1.0,
            )

            # ---- W pass: fwd scan, bwd scan chained on fwd, then square -----
            nc.vector.tensor_tensor_scan(
                out=WF[0:RV, :], data0=ONES[0:RV, :], data1=F[0:RV, :],
                initial=INF_I, op0=ALU.add, op1=ALU.min,
            )
            nc.vector.tensor_tensor_scan(
                out=WB[0:RV, ::-1], data0=ONES[0:RV, ::-1], data1=WF[0:RV, ::-1],
                initial=INF_I, op0=ALU.add, op1=ALU.min,
            )
            A3 = A[:].rearrange("p (h w) -> p h w", w=WW)
            B3 = Bt[:].rearrange("p (h w) -> p h w", w=WW)
            nc.scalar.activation(
                A3[0:RV, :, :],
                WB[:].rearrange("p (h w) -> p h w", w=WPAD)[0:RV, :, 0:WW],
                ACT.Square,
            )

            # ---- H pass: A -> Bt, 6 truncated min-plus shifts ---------------
            nc.vector.scalar_tensor_tensor(
                out=B3[0:RV, 1:HH, :], in0=A3[0:RV, 0 : HH - 1, :], scalar=1.0,
                in1=A3[0:RV, 1:HH, :], op0=ALU.add, op1=ALU.min,
            )
            nc.vector.tensor_copy(B3[0:RV, 0:1, :], A3[0:RV, 0:1, :])
            nc.vector.scalar_tensor_tensor(
                out=B3[0:RV, 0 : HH - 1, :], in0=A3[0:RV, 1:HH, :], scalar=1.0,
                in1=B3[0:RV, 0 : HH - 1, :], op0=ALU.add, op1=ALU.min,
            )
            for s in (2, 3):
                s2 = float(s * s)
                nc.vector.scalar_tensor_tensor(
                    out=B3[0:RV, s:HH, :], in0=A3[0:RV, 0 : HH - s, :],
                    scalar=s2, in1=B3[0:RV, s:HH, :], op0=ALU.add, op1=ALU.min,
                )
                nc.vector.scalar_tensor_tensor(
                    out=B3[0:RV, 0 : HH - s, :], in0=A3[0:RV, s:HH, :],
                    scalar=s2, in1=B3[0:RV, 0 : HH - s, :],
                    op0=ALU.add, op1=ALU.min,
                )

            # ---- D pass: Bt -> A via partition-shifted DMA copies -----------
            shifts = [(1, 1), (1, -1), (2, 1), (2, -1), (3, 1), (3, -1)]
            for j, (s, sign) in enumerate(shifts):
                fs = FS[j]
                if sign > 0:
                    nc.sync.dma_start(fs[s:RV, :], Bt[0 : RV - s, :])
                else:
                    nc.sync.dma_start(fs[0 : RV - s, :], Bt[s:RV, :])
            nc.vector.scalar_tensor_tensor(
                out=A[0:RV, :], in0=FS[0][0:RV, :], scalar=1.0,
                in1=Bt[0:RV, :], op0=ALU.add, op1=ALU.min,
            )
            for j, (s, sign) in enumerate(shifts):
                if j == 0:
                    continue
                nc.vector.scalar_tensor_tensor(
                    out=A[0:RV, :], in0=FS[j][0:RV, :], scalar=float(s * s),
                    in1=A[0:RV, :], op0=ALU.add, op1=ALU.min,
                )

            # ---- |sdf| = sqrt(g_pos) + sqrt(g_neg) --------------------------
            nc.scalar.activation(SQ[0:RV, :], A[0:RV, :], ACT.Sqrt)
            n0 = 0
            while n0 < PLANE:
                nn = min(512, PLANE - n0)
                nc.tensor.matmul(
                    PS[:, n0 : n0 + nn], PM[:], SQ[:, n0 : n0 + nn],
                    start=True, stop=True,
                )
                n0 += nn

            # ---- softmax weight for class c (slot 0 after host permute) -----
            nc.scalar.activation(SME[0:RV, :], SM[0:RV, :], ACT.Exp)
            nc.vector.tensor_tensor(
                DN[0:RV, :], SME[0:RV, 0:1152], SME[0:RV, 1152:2304], ALU.add
            )
            nc.vector.tensor_tensor(
                DN[0:RV, :], DN[0:RV, :], SME[0:RV, 2304:3456], ALU.add
            )
            nc.vector.tensor_tensor(
                DN[0:RV, :], DN[0:RV, :], SME[0:RV, 3456:4608], ALU.add
            )
            nc.vector.reciprocal_approx_fast(out=RC[0:RV, :], in_=DN[0:RV, :])
            nc.vector.tensor_tensor(
                WT[0:RV, :], SME[0:RV, 0:1152], RC[0:RV, :], ALU.mult
            )

            # ---- partial[d] = sum |sdf| * w_c  (TT mults + Act accum) -------
            nc.vector.tensor_tensor(SCR[:], PS[:, 0:1152], WT[0:48, :], ALU.mult)
            nc.scalar.activation(SCR[:], SCR[:], ACT.Copy, accum_out=ACO[:, 0:1])
            nc.vector.tensor_tensor(
                SCR2[:], PS[:, 1152:2304], WT[64:RV, :], ALU.mult
            )
            nc.scalar.activation(SCR2[:], SCR2[:], ACT.Copy, accum_out=ACO[:, 1:2])
            nc.sync.dma_start(out_d[:], ACO[:])

    nc.compile()
    return nc


def kernel(pred, target):
    pred = np.ascontiguousarray(np.asarray(pred), dtype=np.float32)
    target = np.asarray(target)

    if pred.shape != (B, C, DD, HH, WW) or target.shape != (B, DD, HH, WW):
        return _reference_fallback(pred, target)

    tgt = target.astype(np.int64)
    masks = []
    has_pos = {}
    for b in range(B):
        for c in range(C):
            m = tgt[b] == c
            has_pos[(b, c)] = bool(m.any())
            if has_pos[(b, c)]:
                masks.append(m)
                mn = ~m
                if mn.any():
                    masks.append(mn)
                else:
                    return _reference_fallback(pred, target)  # class fills volume

    if _certified_shift_bound(masks) > S_DEV:
        return _reference_fallback(pred, target)

    _ensure_paths()
    from concourse.bass_utils import run_bass_kernel_spmd

    if 0 not in _nc_cache:
        _nc_cache[0] = _build_nc()
    nc = _nc_cache[0]

    NP = 128

    ones = np.ones((NP, NFP), np.int16)
    ones[:, WW::WPAD] = int(INF_I)
    pairmat = np.zeros((NP, 48), np.float32)
    pairmat[np.arange(48), np.arange(48)] = 1.0
    pairmat[64 + np.arange(48), np.arange(48)] = 1.0

    in_maps = []
    for k in range(N_CORES):
        b, c = divmod(k, C)
        t16 = tgt[b].reshape(DD, PLANE).astype(np.int16)
        T = np.empty((128, PLANE), np.int16)
        T[0:48] = t16
        T[48:64] = 4  # sentinel: (4-c)^2 * 1024 >= 1024 isolates the blocks
        T[64:112] = t16
        T[112:128] = 4
        cvn = np.full((NP, 1), -32.0 * c, np.float32)
        cvc = np.full((NP, 1), INF_SEED, np.float32)  # Relu bias
        perm = [c] + [j for j in range(C) if j != c]
        # [h2*64+d, (c', h%24, w)] layout with class c in slot 0
        sm = (
            pred[b][perm]
            .reshape(C, DD, 2, 24, WW)
            .transpose(2, 1, 0, 3, 4)
            .reshape(2, 48, 4 * 1152)
        )
        in_maps.append(
            {
                "tgt": T,
                "cvn": cvn,
                "cvc": cvc,
                "ones": ones,
                "predsm": np.ascontiguousarray(sm),
                "pairmat": pairmat,
            }
        )

    trace = bool(os.environ.get("BOUNDARY_KERNEL_TRACE"))
    if trace:
        import importlib.util

        if importlib.util.find_spec("antenv.axon_hooks") is None:
            trace = False
    res = run_bass_kernel_spmd(nc, in_maps, list(range(N_CORES)), trace=trace)
    global LAST_RESULTS
    LAST_RESULTS = res

    total = 0.0
    for k in range(N_CORES):
        b, c = divmod(k, C)
        if has_pos[(b, c)]:
            total += float(res.results[k]["out"].astype(np.float64).sum())
    return np.float32(total / (B * C * NVOX))


if __name__ == "__main__":
    import reference

    inputs = reference.setup_inputs()
    out = kernel(**{k: np.asarray(v) for k, v in inputs.items()})
    print("kernel out:", out)


# revision 10
# speedup vs baseline: 2.6841x; 2.4892x over previous
"""Trainium2 Bass kernel for nn_BoundaryLoss: mean(|softmax(pred) * SDF(onehot(target))|).

Strategy (8 NeuronCores, SPMD, one (b, c) pair per core):
  - Exact 3D squared EDT of the class-c mask (pos) and complement (neg) via
    separable passes. |sdf| = sqrt(g_pos) + sqrt(g_neg) since exactly one of
    the two is zero at every voxel.
  - W pass: two tensor_tensor_scan ops (fwd chamfer, then bwd chamfer chained
    on the fwd result) give the exact 1D L1 seed distance per row; one Square
    activation turns it into the squared EDT. INF pads between rows stop the
    recurrence from leaking across (h) rows.
  - H pass: 6 scalar_tensor_tensor min-plus shift updates (s = 1..3, both
    directions, truncation certified exact on the host). First op fused with
    the pass-init copy.
  - D pass: same, with the partition-axis shifts materialized by SBUF->SBUF
    DMA copies (borders filled from an INF tile; compute ops never straddle
    non-aligned partition starts).
  - Softmax: pred DMA'd in layout [h2*64+d, (c, h%24, w)] with class c
    permuted to slot 0; exp on the scalar engine; 4-way denominator adds,
    reciprocal_approx_fast and the weight multiply on DVE; two fused
    tensor_tensor_reduce ops against the PE pair-sum produce 48 partials.
  - Host sums the 8x48 partials, applies the has_pos gate and the mean factor.

Engine budget: DVE ~42us is the critical path; Act (init/exp/sqrt), pool
(neg-init, pad memsets), PE (pair-sum matmul) and the DMA queue all hide
under it.
"""

import os
import sys

import numpy as np

B, C, DD, HH, WW = 2, 4, 48, 48, 48
PLANE = HH * WW  # 2304
WPAD = WW + 1  # 49: w row + INF pad column
NFP = HH * WPAD  # 2352 padded free size
NVOX = DD * PLANE
S_MAX = 16
S_DEV = 3  # device kernel is built for shift radius 3; larger -> fallback
N_CORES = 8
RV = 112  # end of valid rows: pos [0:48) | gap [48:64) | neg [64:112)
INF_I = 30000.0  # scan/pad infinity (int16 domain)
INF_SEED = 1024.0  # neg-mask seed infinity (= 32^2, matches Square scaling)

_nc_cache = {}
LAST_RESULTS = None  # test harness introspection


def _ensure_paths():
    for p in ("/opt/trn_rl_repo",):
        if os.path.isdir(p) and p not in sys.path:
            sys.path.insert(0, p)


def _edt_sq_trunc_np(f0, S):
    """Truncated-shift separable squared EDT (numpy, int32). Mirrors the device
    algorithm; used for the shift-bound certification and the fallback path."""
    f = f0.astype(np.int32)
    for ax in (2, 1, 0):
        g = f.copy()
        for s in range(1, S + 1):
            s2 = s * s
            sl_out = [slice(None)] * 3
            sl_in = [slice(None)] * 3
            sl_out[ax] = slice(s, None)
            sl_in[ax] = slice(None, -s)
            np.minimum(g[tuple(sl_out)], f[tuple(sl_in)] + s2, out=g[tuple(sl_out)])
            sl_out[ax] = slice(None, -s)
            sl_in[ax] = slice(s, None)
            np.minimum(g[tuple(sl_out)], f[tuple(sl_in)] + s2, out=g[tuple(sl_out)])
        f = g
    return f


def _certified_shift_bound(masks):
    """Smallest S such that the S-truncated separable EDT is provably exact for
    every seed mask in `masks` (max truncated distance <= S certifies that no
    winning chain was cut off). The device kernel's exact-W variant is then
    also exact, since relaxing one pass can only move the result toward the
    true EDT."""
    for S in range(1, S_MAX + 1):
        worst = 0
        for m in masks:
            f0 = np.where(m, 0, 30000).astype(np.int16)
            g = _edt_sq_trunc_np(f0, S)
            worst = max(worst, int(np.ceil(np.sqrt(float(g.max())))))
        if worst <= S:
            return S
    return S_MAX + 1


def _reference_fallback(pred, target):
    """Exact numpy replica of the reference for pathological inputs the device
    path does not cover (wrong shapes, class filling a volume, S > S_DEV)."""
    INF = 1e9
    pred = np.asarray(pred, np.float32)
    target = np.asarray(target)
    b_, c_ = pred.shape[0], pred.shape[1]
    n = np.arange(pred.shape[-1])

    def minplus(f):
        d2 = ((n[:, None] - n[None, :]) ** 2).astype(np.float32)
        return (f[..., None, :] + d2).min(axis=-1)

    def edt(src):
        f = np.where(src, 0.0, INF).astype(np.float32)
        for ax in (-3, -2, -1):
            f = np.moveaxis(minplus(np.moveaxis(f, ax, -1)), -1, ax)
        return np.sqrt(f)

    e = np.exp(pred - pred.max(axis=1, keepdims=True))
    sm = e / e.sum(axis=1, keepdims=True)
    total = 0.0
    for b in range(b_):
        for c in range(c_):
            pos = target[b] == c
            if not pos.any():
                continue
            sdf = edt(pos) - edt(~pos)
            total += float(np.abs(sm[b, c] * sdf).sum(dtype=np.float64))
    return np.float32(total / pred.size)


def _build_nc():
    _ensure_paths()
    import concourse.tile as tile
    from concourse import bacc, mybir

    i16 = mybir.dt.int16
    f32 = mybir.dt.float32
    bf16 = mybir.dt.bfloat16
    ALU = mybir.AluOpType
    ACT = mybir.ActivationFunctionType

    NP = 128

    nc = bacc.Bacc("TRN2", target_bir_lowering=False, debug=False)

    tgt_d = nc.dram_tensor("tgt", [128, PLANE], i16, kind="ExternalInput")
    cvn_d = nc.dram_tensor("cvn", [NP, 1], f32, kind="ExternalInput")
    cvc_d = nc.dram_tensor("cvc", [NP, 1], f32, kind="ExternalInput")
    ones_d = nc.dram_tensor("ones", [NP, NFP], i16, kind="ExternalInput")
    sm_d = nc.dram_tensor("predsm", [2, 48, 4 * 1152], f32, kind="ExternalInput")
    pm_d = nc.dram_tensor("pairmat", [NP, 48], bf16, kind="ExternalInput")
    psh_d = nc.dram_tensor("pshift", [NP, 6 * NP], bf16, kind="ExternalInput")
    out_d = nc.dram_tensor("out", [48, 2], f32, kind="ExternalOutput")

    with tile.TileContext(nc) as tc:
        with (
            tc.tile_pool(name="main", bufs=1) as pool,
            tc.tile_pool(name="psh", bufs=2, space="PSUM") as psh_pool,
            tc.tile_pool(name="psq", bufs=1, space="PSUM") as psq_pool,
        ):
            T64 = pool.tile([NP, PLANE], i16, tag="T64")
            CVN = pool.tile([NP, 1], f32, tag="CVN")
            CVC = pool.tile([NP, 1], f32, tag="CVC")
            ONES = pool.tile([NP, NFP], i16, tag="ONES")
            SM = pool.tile([NP, 4 * 1152], f32, tag="SM")
            SME = pool.tile([NP, 4 * 1152], f32, tag="SME")
            PM = pool.tile([NP, 48], bf16, tag="PM")
            PSH = pool.tile([NP, 6 * NP], bf16, tag="PSH")
            F = pool.tile([NP, NFP], i16, tag="F")
            WF = pool.tile([NP, NFP], i16, tag="WF")
            WB = pool.tile([NP, NFP], i16, tag="WB")
            A = pool.tile([NP, PLANE], bf16, tag="A")
            Bt = pool.tile([NP, PLANE], bf16, tag="B")
            SQ = pool.tile([NP, PLANE], bf16, tag="SQ")
            DN = pool.tile([NP, 1152], f32, tag="DN")
            RC = pool.tile([NP, 1152], f32, tag="RC")
            WT = pool.tile([NP, 1152], f32, tag="WT")
            SCR = pool.tile([48, 1152], f32, tag="SCR")
            SCR2 = pool.tile([48, 1152], f32, tag="SCR2")
            ACO = pool.tile([48, 2], f32, tag="ACO")


            # ---- input DMAs -------------------------------------------------
            nc.sync.dma_start(T64[:], tgt_d[:])
            nc.sync.dma_start(CVN[:], cvn_d[:])
            nc.sync.dma_start(CVC[:], cvc_d[:])
            nc.sync.dma_start(ONES[:], ones_d[:])
            nc.sync.dma_start(SM[0:48, :], sm_d[0])
            nc.sync.dma_start(SM[64:RV, :], sm_d[1])
            nc.sync.dma_start(PM[:], pm_d[:])
            nc.sync.dma_start(PSH[:], psh_d[:])

            Fp = F[:].rearrange("p (h w) -> p h w", w=WPAD)

            # pool: pad columns of F to INF, SQ tail to 0, neg-mask init
            nc.gpsimd.memset(Fp[0:RV, :, WW : WW + 1], INF_I)
            nc.gpsimd.memset(SQ[96:NP, :], 0.0)  # sqrt later rewrites [96:RV)
            nc.gpsimd.memset(Bt[96:NP, :], 100.0)  # rows [96:RV) rewritten by H

            # Act: pos-mask init f = (32*(t-c))^2 over pos+gap rows (gap rows
            # carry the t=4 sentinel -> >= INF_SEED, isolating the blocks)
            nc.scalar.activation(
                Fp[0:64, :, 0:WW],
                T64[0:64, :].rearrange("p (h w) -> p h w", w=WW),
                ACT.Square, bias=CVN[0:64, :], scale=32.0,
            )
            nc.scalar.activation(
                Fp[64:RV, :, 0:WW], Fp[0:48, :, 0:WW],
                ACT.Relu, bias=CVC[64:RV, :], scale=-1.0,
            )

            # ---- W pass: fwd scan, bwd scan chained on fwd, then square -----
            nc.vector.tensor_tensor_scan(
                out=WF[0:RV, :], data0=ONES[0:RV, :], data1=F[0:RV, :],
                initial=INF_I, op0=ALU.add, op1=ALU.min,
            )
            nc.vector.tensor_tensor_scan(
                out=WB[0:RV, ::-1], data0=ONES[0:RV, ::-1], data1=WF[0:RV, ::-1],
                initial=INF_I, op0=ALU.add, op1=ALU.min,
            )
            A3 = A[:].rearrange("p (h w) -> p h w", w=WW)
            B3 = Bt[:].rearrange("p (h w) -> p h w", w=WW)
            nc.scalar.activation(
                A3[0:RV, :, :],
                WB[:].rearrange("p (h w) -> p h w", w=WPAD)[0:RV, :, 0:WW],
                ACT.Square,
            )

            # ---- H pass: A -> Bt, 6 truncated min-plus shifts ---------------
            nc.vector.scalar_tensor_tensor(
                out=B3[0:RV, 1:HH, :], in0=A3[0:RV, 0 : HH - 1, :], scalar=1.0,
                in1=A3[0:RV, 1:HH, :], op0=ALU.add, op1=ALU.min,
            )
            nc.vector.tensor_copy(B3[0:RV, 0:1, :], A3[0:RV, 0:1, :])
            nc.vector.scalar_tensor_tensor(
                out=B3[0:RV, 0 : HH - 1, :], in0=A3[0:RV, 1:HH, :], scalar=1.0,
                in1=B3[0:RV, 0 : HH - 1, :], op0=ALU.add, op1=ALU.min,
            )
            for s in (2, 3):
                s2 = float(s * s)
                nc.vector.scalar_tensor_tensor(
                    out=B3[0:RV, s:HH, :], in0=A3[0:RV, 0 : HH - s, :],
                    scalar=s2, in1=B3[0:RV, s:HH, :], op0=ALU.add, op1=ALU.min,
                )
                nc.vector.scalar_tensor_tensor(
                    out=B3[0:RV, 0 : HH - s, :], in0=A3[0:RV, s:HH, :],
                    scalar=s2, in1=B3[0:RV, 0 : HH - s, :],
                    op0=ALU.add, op1=ALU.min,
                )

            # ---- D pass: partition shifts via PE 0/1-matmuls into PSUM ------
            shifts = [(1, 1), (1, -1), (2, 1), (2, -1), (3, 1), (3, -1)]
            for j, (s, sign) in enumerate(shifts):
                stat = PSH[:, j * NP : (j + 1) * NP]
                for piece in range(3):
                    pj = psh_pool.tile([NP, 768], f32, tag="sh", name="pj")
                    h0 = piece * 768
                    c0 = 0
                    while c0 < 768:
                        nn = min(512, 768 - c0)
                        nc.tensor.matmul(
                            pj[:, c0 : c0 + nn], stat,
                            Bt[:, h0 + c0 : h0 + c0 + nn],
                            start=True, stop=True,
                        )
                        c0 += nn
                    src_prev = Bt if j == 0 else A
                    nc.vector.scalar_tensor_tensor(
                        out=A[0:RV, h0 : h0 + 768], in0=pj[0:RV, :],
                        scalar=float(s * s),
                        in1=src_prev[0:RV, h0 : h0 + 768],
                        op0=ALU.add, op1=ALU.min,
                    )

            # ---- |sdf| = sqrt(g_pos) + sqrt(g_neg) --------------------------
            nc.scalar.activation(SQ[0:RV, :], A[0:RV, :], ACT.Sqrt)

            # ---- softmax weight for class c (slot 0 after host permute) -----
            nc.scalar.activation(SME[0:RV, :], SM[0:RV, :], ACT.Exp)
            nc.vector.tensor_tensor(
                DN[0:RV, :], SME[0:RV, 0:1152], SME[0:RV, 1152:2304], ALU.add
            )
            nc.vector.tensor_tensor(
                DN[0:RV, :], DN[0:RV, :], SME[0:RV, 2304:3456], ALU.add
            )
            nc.vector.tensor_tensor(
                DN[0:RV, :], DN[0:RV, :], SME[0:RV, 3456:4608], ALU.add
            )
            nc.vector.reciprocal_approx_fast(out=RC[0:RV, :], in_=DN[0:RV, :])
            nc.vector.tensor_tensor(
                WT[0:RV, :], SME[0:RV, 0:1152], RC[0:RV, :], ALU.mult
            )

            # ---- partial[d] = sum |sdf| * w_c  (TT mults + Act accum) -------
            for half, scr, wrows, aco in (
                (0, SCR, WT[0:48, :], ACO[:, 0:1]),
                (1, SCR2, WT[64:RV, :], ACO[:, 1:2]),
            ):
                ps = psq_pool.tile([48, 1152], f32, tag="pp", name="pshalf")
                h0 = half * 1152
                c0 = 0
                while c0 < 1152:
                    nn = min(512, 1152 - c0)
                    nc.tensor.matmul(
                        ps[:, c0 : c0 + nn], PM[:], SQ[:, h0 + c0 : h0 + c0 + nn],
                        start=True, stop=True,
                    )
                    c0 += nn
                nc.vector.tensor_tensor(scr[:], ps[:], wrows, ALU.mult)
                nc.scalar.activation(scr[:], scr[:], ACT.Copy, accum_out=aco)
            nc.sync.dma_start(out_d[:], ACO[:])

    nc.compile()
    return nc


def kernel(pred, target):
    pred = np.ascontiguousarray(np.asarray(pred), dtype=np.float32)
    target = np.asarray(target)

    if pred.shape != (B, C, DD, HH, WW) or target.shape != (B, DD, HH, WW):
        return _reference_fallback(pred, target)

    tgt = target.astype(np.int64)
    masks = []
    has_pos = {}
    for b in range(B):
        for c in range(C):
            m = tgt[b] == c
            has_pos[(b, c)] = bool(m.any())
            if has_pos[(b, c)]:
                masks.append(m)
                mn = ~m
                if mn.any():
                    masks.append(mn)
                else:
                    return _reference_fallback(pred, target)  # class fills volume

    if _certified_shift_bound(masks) > S_DEV:
        return _reference_fallback(pred, target)

    _ensure_paths()
    from concourse.bass_utils import run_bass_kernel_spmd

    if 0 not in _nc_cache:
        _nc_cache[0] = _build_nc()
    nc = _nc_cache[0]

    NP = 128

    ones = np.ones((NP, NFP), np.int16)
    ones[:, WW::WPAD] = int(INF_I)
    try:
        import ml_dtypes

        bf = ml_dtypes.bfloat16
    except ImportError:  # pragma: no cover
        bf = np.float32
    pairmat = np.zeros((NP, 48), np.float32)
    pairmat[np.arange(48), np.arange(48)] = 1.0
    pairmat[64 + np.arange(48), np.arange(48)] = 1.0
    pairmat = pairmat.astype(bf)
    RVL = RV
    pshift = np.zeros((NP, 6 * NP), np.float32)
    for j, (s, sign) in enumerate([(1, 1), (1, -1), (2, 1), (2, -1), (3, 1), (3, -1)]):
        m = pshift[:, j * NP : (j + 1) * NP]
        for p in range(RVL):
            q = p - s if sign > 0 else p + s
            if 0 <= q < RVL:
                m[q, p] = 1.0
            else:
                m[127, p] = 1.0  # INF row
    pshift = pshift.astype(bf)

    in_maps = []
    for k in range(N_CORES):
        b, c = divmod(k, C)
        t16 = tgt[b].reshape(DD, PLANE).astype(np.int16)
        T = np.empty((128, PLANE), np.int16)
        T[0:48] = t16
        T[48:64] = 4  # sentinel: (4-c)^2 * 1024 >= 1024 isolates the blocks
        T[64:112] = t16
        T[112:128] = 4
        cvn = np.full((NP, 1), -32.0 * c, np.float32)
        cvc = np.full((NP, 1), INF_SEED, np.float32)  # Relu bias
        perm = [c] + [j for j in range(C) if j != c]
        # [h2*64+d, (c', h%24, w)] layout with class c in slot 0
        sm = (
            pred[b][perm]
            .reshape(C, DD, 2, 24, WW)
            .transpose(2, 1, 0, 3, 4)
            .reshape(2, 48, 4 * 1152)
        )
        in_maps.append(
            {
                "tgt": T,
                "cvn": cvn,
                "cvc": cvc,
                "ones": ones,
                "predsm": np.ascontiguousarray(sm),
                "pairmat": pairmat,
                "pshift": pshift,
            }
        )

    trace = bool(os.environ.get("BOUNDARY_KERNEL_TRACE"))
    if trace:
        import importlib.util

        if importlib.util.find_spec("antenv.axon_hooks") is None:
            trace = False
    res = run_bass_kernel_spmd(nc, in_maps, list(range(N_CORES)), trace=trace)
    global LAST_RESULTS
    LAST_RESULTS = res

    total = 0.0
    for k in range(N_CORES):
        b, c = divmod(k, C)
        if has_pos[(b, c)]:
            total += float(res.results[k]["out"].astype(np.float64).sum())
    return np.float32(total / (B * C * NVOX))


if __name__ == "__main__":
    import reference

    inputs = reference.setup_inputs()
    out = kernel(**{k: np.asarray(v) for k, v in inputs.items()})
    print("kernel out:", out)


# revision 11
# speedup vs baseline: 2.8471x; 1.0607x over previous
"""Trainium2 Bass kernel for nn_BoundaryLoss: mean(|softmax(pred) * SDF(onehot(target))|).

Strategy (8 NeuronCores, SPMD, one (b, c) pair per core):
  - Exact 3D squared EDT of the class-c mask (pos) and complement (neg) via
    separable passes. |sdf| = sqrt(g_pos) + sqrt(g_neg) since exactly one of
    the two is zero at every voxel.
  - W pass: two tensor_tensor_scan ops (fwd chamfer, then bwd chamfer chained
    on the fwd result) give the exact 1D L1 seed distance per row; one Square
    activation turns it into the squared EDT. INF pads between rows stop the
    recurrence from leaking across (h) rows.
  - H pass: 6 scalar_tensor_tensor min-plus shift updates (s = 1..3, both
    directions, truncation certified exact on the host). First op fused with
    the pass-init copy.
  - D pass: same, with the partition-axis shifts materialized by SBUF->SBUF
    DMA copies (borders filled from an INF tile; compute ops never straddle
    non-aligned partition starts).
  - Softmax: pred DMA'd in layout [h2*64+d, (c, h%24, w)] with class c
    permuted to slot 0; exp on the scalar engine; 4-way denominator adds,
    reciprocal_approx_fast and the weight multiply on DVE; two fused
    tensor_tensor_reduce ops against the PE pair-sum produce 48 partials.
  - Host sums the 8x48 partials, applies the has_pos gate and the mean factor.

Engine budget: DVE ~42us is the critical path; Act (init/exp/sqrt), pool
(neg-init, pad memsets), PE (pair-sum matmul) and the DMA queue all hide
under it.
"""

import os
import sys

import numpy as np

B, C, DD, HH, WW = 2, 4, 48, 48, 48
PLANE = HH * WW  # 2304
WPAD = WW + 1  # 49: w row + INF pad column
NFP = HH * WPAD  # 2352 padded free size
NVOX = DD * PLANE
S_MAX = 16
S_DEV = 3  # device kernel is built for shift radius 3; larger -> fallback
N_CORES = 8
RV = 112  # end of valid rows: pos [0:48) | gap [48:64) | neg [64:112)
INF_I = 30000.0  # scan/pad infinity (int16 domain)
INF_SEED = 1024.0  # neg-mask seed infinity (= 32^2, matches Square scaling)

_nc_cache = {}
LAST_RESULTS = None  # test harness introspection


def _ensure_paths():
    for p in ("/opt/trn_rl_repo",):
        if os.path.isdir(p) and p not in sys.path:
            sys.path.insert(0, p)


def _edt_sq_trunc_np(f0, S):
    """Truncated-shift separable squared EDT (numpy, int32). Mirrors the device
    algorithm; used for the shift-bound certification and the fallback path."""
    f = f0.astype(np.int32)
    for ax in (2, 1, 0):
        g = f.copy()
        for s in range(1, S + 1):
            s2 = s * s
            sl_out = [slice(None)] * 3
            sl_in = [slice(None)] * 3
            sl_out[ax] = slice(s, None)
            sl_in[ax] = slice(None, -s)
            np.minimum(g[tuple(sl_out)], f[tuple(sl_in)] + s2, out=g[tuple(sl_out)])
            sl_out[ax] = slice(None, -s)
            sl_in[ax] = slice(s, None)
            np.minimum(g[tuple(sl_out)], f[tuple(sl_in)] + s2, out=g[tuple(sl_out)])
        f = g
    return f


def _certified_shift_bound(masks):
    """Smallest S such that the S-truncated separable EDT is provably exact for
    every seed mask in `masks` (max truncated distance <= S certifies that no
    winning chain was cut off). The device kernel's exact-W variant is then
    also exact, since relaxing one pass can only move the result toward the
    true EDT."""
    for S in range(1, S_MAX + 1):
        worst = 0
        for m in masks:
            f0 = np.where(m, 0, 30000).astype(np.int16)
            g = _edt_sq_trunc_np(f0, S)
            worst = max(worst, int(np.ceil(np.sqrt(float(g.max())))))
        if worst <= S:
            return S
    return S_MAX + 1


def _reference_fallback(pred, target):
    """Exact numpy replica of the reference for pathological inputs the device
    path does not cover (wrong shapes, class filling a volume, S > S_DEV)."""
    INF = 1e9
    pred = np.asarray(pred, np.float32)
    target = np.asarray(target)
    b_, c_ = pred.shape[0], pred.shape[1]
    n = np.arange(pred.shape[-1])

    def minplus(f):
        d2 = ((n[:, None] - n[None, :]) ** 2).astype(np.float32)
        return (f[..., None, :] + d2).min(axis=-1)

    def edt(src):
        f = np.where(src, 0.0, INF).astype(np.float32)
        for ax in (-3, -2, -1):
            f = np.moveaxis(minplus(np.moveaxis(f, ax, -1)), -1, ax)
        return np.sqrt(f)

    e = np.exp(pred - pred.max(axis=1, keepdims=True))
    sm = e / e.sum(axis=1, keepdims=True)
    total = 0.0
    for b in range(b_):
        for c in range(c_):
            pos = target[b] == c
            if not pos.any():
                continue
            sdf = edt(pos) - edt(~pos)
            total += float(np.abs(sm[b, c] * sdf).sum(dtype=np.float64))
    return np.float32(total / pred.size)


def _build_nc():
    _ensure_paths()
    import concourse.tile as tile
    from concourse import bacc, mybir

    i16 = mybir.dt.int16
    f32 = mybir.dt.float32
    bf16 = mybir.dt.bfloat16
    ALU = mybir.AluOpType
    ACT = mybir.ActivationFunctionType

    NP = 128

    nc = bacc.Bacc("TRN2", target_bir_lowering=False, debug=False)

    tgt_d = nc.dram_tensor("tgt", [128, PLANE], i16, kind="ExternalInput")
    cvn_d = nc.dram_tensor("cvn", [NP, 1], f32, kind="ExternalInput")
    cvc_d = nc.dram_tensor("cvc", [NP, 1], f32, kind="ExternalInput")
    ones_d = nc.dram_tensor("ones", [NP, NFP], i16, kind="ExternalInput")
    sm_d = nc.dram_tensor("predsm", [2, 48, 4 * 1152], f32, kind="ExternalInput")
    pm_d = nc.dram_tensor("pairmat", [NP, 48], bf16, kind="ExternalInput")
    psh_d = nc.dram_tensor("pshift", [NP, 6 * NP], bf16, kind="ExternalInput")
    out_d = nc.dram_tensor("out", [48, 4], f32, kind="ExternalOutput")

    with tile.TileContext(nc) as tc:
        with (
            tc.tile_pool(name="main", bufs=1) as pool,
            tc.tile_pool(name="psh", bufs=2, space="PSUM") as psh_pool,
            tc.tile_pool(name="psq", bufs=1, space="PSUM") as psq_pool,
        ):
            T64 = pool.tile([NP, PLANE], i16, tag="T64")
            CVN = pool.tile([NP, 1], f32, tag="CVN")
            CVC = pool.tile([NP, 1], f32, tag="CVC")
            ONES = pool.tile([NP, NFP], i16, tag="ONES")
            SM = pool.tile([NP, 4 * 1152], f32, tag="SM")
            SME = pool.tile([NP, 4 * 1152], f32, tag="SME")
            PM = pool.tile([NP, 48], bf16, tag="PM")
            PSH = pool.tile([NP, 6 * NP], bf16, tag="PSH")
            F = pool.tile([NP, NFP], i16, tag="F")
            WF = pool.tile([NP, NFP], i16, tag="WF")
            WB = pool.tile([NP, NFP], i16, tag="WB")
            A = pool.tile([NP, PLANE], bf16, tag="A")
            Bt = pool.tile([NP, PLANE], bf16, tag="B")
            SQ = pool.tile([NP, PLANE], bf16, tag="SQ")
            DN = pool.tile([NP, 1152], f32, tag="DN")
            RC = pool.tile([NP, 1152], f32, tag="RC")
            WT = pool.tile([NP, 1152], f32, tag="WT")
            SCR = pool.tile([48, 1152], f32, tag="SCR")
            SCR2 = pool.tile([48, 1152], f32, tag="SCR2")
            ACO = pool.tile([48, 4], f32, tag="ACO")


            DU = pool.tile([1, 1], f32, tag="DU")
            nc.vector.memset(DU[:], 4.0)
            nc.scalar.activation(DU[:], DU[:], ACT.Square)  # preload act table 0

            # ---- input DMAs -------------------------------------------------
            nc.sync.dma_start(T64[:], tgt_d[:])
            nc.sync.dma_start(CVN[:], cvn_d[:])
            nc.sync.dma_start(CVC[:], cvc_d[:])
            nc.sync.dma_start(ONES[:], ones_d[:])
            nc.sync.dma_start(SM[0:48, :], sm_d[0])
            nc.sync.dma_start(SM[64:RV, :], sm_d[1])
            nc.sync.dma_start(PM[:], pm_d[:])
            nc.sync.dma_start(PSH[:], psh_d[:])

            Fp = F[:].rearrange("p (h w) -> p h w", w=WPAD)

            # pool: pad columns of F to INF, SQ tail to 0, neg-mask init
            nc.gpsimd.memset(Fp[0:RV, :, WW : WW + 1], INF_I)
            nc.gpsimd.memset(SQ[96:NP, :], 0.0)  # sqrt later rewrites [96:RV)
            nc.gpsimd.memset(Bt[96:NP, :], 100.0)  # rows [96:RV) rewritten by H

            # Act: pos-mask init f = (32*(t-c))^2 over pos+gap rows (gap rows
            # carry the t=4 sentinel -> >= INF_SEED, isolating the blocks)
            nc.scalar.activation(
                Fp[0:64, :, 0:WW],
                T64[0:64, :].rearrange("p (h w) -> p h w", w=WW),
                ACT.Square, bias=CVN[0:64, :], scale=32.0,
            )
            nc.vector.tensor_scalar(
                out=Fp[64:RV, :, 0:WW],
                in0=T64[64:RV, :].rearrange("p (h w) -> p h w", w=WW),
                scalar1=CVC[64:RV, :], scalar2=INF_SEED,
                op0=ALU.is_equal, op1=ALU.mult,
            )

            # ---- W pass: fwd scan, bwd scan chained on fwd, then square -----
            nc.vector.tensor_tensor_scan(
                out=WF[0:RV, :], data0=ONES[0:RV, :], data1=F[0:RV, :],
                initial=INF_I, op0=ALU.add, op1=ALU.min,
            )
            nc.vector.tensor_tensor_scan(
                out=WB[0:RV, ::-1], data0=ONES[0:RV, ::-1], data1=WF[0:RV, ::-1],
                initial=INF_I, op0=ALU.add, op1=ALU.min,
            )
            A3 = A[:].rearrange("p (h w) -> p h w", w=WW)
            B3 = Bt[:].rearrange("p (h w) -> p h w", w=WW)
            nc.scalar.activation(
                A3[0:RV, :, :],
                WB[:].rearrange("p (h w) -> p h w", w=WPAD)[0:RV, :, 0:WW],
                ACT.Square,
            )

            # ---- H pass: A -> Bt, 6 truncated min-plus shifts ---------------
            nc.vector.scalar_tensor_tensor(
                out=B3[0:RV, 1:HH, :], in0=A3[0:RV, 0 : HH - 1, :], scalar=1.0,
                in1=A3[0:RV, 1:HH, :], op0=ALU.add, op1=ALU.min,
            )
            nc.vector.tensor_copy(B3[0:RV, 0:1, :], A3[0:RV, 0:1, :])
            nc.vector.scalar_tensor_tensor(
                out=B3[0:RV, 0 : HH - 1, :], in0=A3[0:RV, 1:HH, :], scalar=1.0,
                in1=B3[0:RV, 0 : HH - 1, :], op0=ALU.add, op1=ALU.min,
            )
            for s in (2, 3):
                s2 = float(s * s)
                nc.vector.scalar_tensor_tensor(
                    out=B3[0:RV, s:HH, :], in0=A3[0:RV, 0 : HH - s, :],
                    scalar=s2, in1=B3[0:RV, s:HH, :], op0=ALU.add, op1=ALU.min,
                )
                nc.vector.scalar_tensor_tensor(
                    out=B3[0:RV, 0 : HH - s, :], in0=A3[0:RV, s:HH, :],
                    scalar=s2, in1=B3[0:RV, 0 : HH - s, :],
                    op0=ALU.add, op1=ALU.min,
                )

            # ---- D pass: partition shifts via PE 0/1-matmuls into PSUM ------
            shifts = [(1, 1), (1, -1), (2, 1), (2, -1), (3, 1), (3, -1)]
            for j, (s, sign) in enumerate(shifts):
                stat = PSH[:, j * NP : (j + 1) * NP]
                for piece in range(2):
                    pj = psh_pool.tile([NP, 1152], f32, tag="sh", name="pj")
                    h0 = piece * 1152
                    c0 = 0
                    while c0 < 1152:
                        nn = min(512, 1152 - c0)
                        nc.tensor.matmul(
                            pj[:, c0 : c0 + nn], stat,
                            Bt[:, h0 + c0 : h0 + c0 + nn],
                            start=True, stop=True,
                        )
                        c0 += nn
                    src_prev = Bt if j == 0 else A
                    nc.vector.scalar_tensor_tensor(
                        out=A[0:RV, h0 : h0 + 1152], in0=pj[0:RV, :],
                        scalar=float(s * s),
                        in1=src_prev[0:RV, h0 : h0 + 1152],
                        op0=ALU.add, op1=ALU.min,
                    )

            # ---- |sdf| = sqrt(g_pos) + sqrt(g_neg) --------------------------
            nc.scalar.activation(SQ[0:RV, 0:1152], A[0:RV, 0:1152], ACT.Sqrt)
            nc.scalar.activation(SQ[0:RV, 1152:2304], A[0:RV, 1152:2304], ACT.Sqrt)

            # ---- softmax weight for class c (slot 0 after host permute) -----
            nc.scalar.activation(SME[0:RV, :], SM[0:RV, :], ACT.Exp)
            nc.scalar.activation(DU[:], DU[:], ACT.Sqrt)  # preload act table 1
            nc.vector.tensor_tensor(
                DN[0:RV, :], SME[0:RV, 0:1152], SME[0:RV, 1152:2304], ALU.add
            )
            nc.vector.tensor_tensor(
                DN[0:RV, :], DN[0:RV, :], SME[0:RV, 2304:3456], ALU.add
            )
            nc.vector.tensor_tensor(
                DN[0:RV, :], DN[0:RV, :], SME[0:RV, 3456:4608], ALU.add
            )
            nc.vector.reciprocal_approx_fast(out=RC[0:RV, :], in_=DN[0:RV, :])
            nc.vector.tensor_tensor(
                WT[0:RV, :], SME[0:RV, 0:1152], RC[0:RV, :], ALU.mult
            )

            # ---- partial[d] = sum |sdf| * w_c  (TT mults + Act accum) -------
            for q in range(4):
                half = q // 2
                wrows = WT[0:48, :] if half == 0 else WT[64:RV, :]
                w0 = (q % 2) * 576
                scr = SCR if q % 2 == 0 else SCR2
                ps = psq_pool.tile([48, 576], f32, tag="pp", name="pshalf")
                h0 = q * 576
                c0 = 0
                while c0 < 576:
                    nn = min(512, 576 - c0)
                    nc.tensor.matmul(
                        ps[:, c0 : c0 + nn], PM[:], SQ[:, h0 + c0 : h0 + c0 + nn],
                        start=True, stop=True,
                    )
                    c0 += nn
                nc.vector.tensor_tensor(
                    scr[:, 0:576], ps[:], wrows[:, w0 : w0 + 576], ALU.mult
                )
                nc.scalar.activation(
                    scr[:, 0:576], scr[:, 0:576], ACT.Copy,
                    accum_out=ACO[:, q : q + 1],
                )
            nc.sync.dma_start(out_d[:], ACO[:])

    nc.compile()
    return nc


def kernel(pred, target):
    pred = np.ascontiguousarray(np.asarray(pred), dtype=np.float32)
    target = np.asarray(target)

    if pred.shape != (B, C, DD, HH, WW) or target.shape != (B, DD, HH, WW):
        return _reference_fallback(pred, target)

    tgt = target.astype(np.int64)
    masks = []
    has_pos = {}
    for b in range(B):
        for c in range(C):
            m = tgt[b] == c
            has_pos[(b, c)] = bool(m.any())
            if has_pos[(b, c)]:
                masks.append(m)
                mn = ~m
                if mn.any():
                    masks.append(mn)
                else:
                    return _reference_fallback(pred, target)  # class fills volume

    if _certified_shift_bound(masks) > S_DEV:
        return _reference_fallback(pred, target)

    _ensure_paths()
    from concourse.bass_utils import run_bass_kernel_spmd

    if 0 not in _nc_cache:
        _nc_cache[0] = _build_nc()
    nc = _nc_cache[0]

    NP = 128

    ones = np.ones((NP, NFP), np.int16)
    ones[:, WW::WPAD] = int(INF_I)
    try:
        import ml_dtypes

        bf = ml_dtypes.bfloat16
    except ImportError:  # pragma: no cover
        bf = np.float32
    pairmat = np.zeros((NP, 48), np.float32)
    pairmat[np.arange(48), np.arange(48)] = 1.0
    pairmat[64 + np.arange(48), np.arange(48)] = 1.0
    pairmat = pairmat.astype(bf)
    RVL = RV
    pshift = np.zeros((NP, 6 * NP), np.float32)
    for j, (s, sign) in enumerate([(1, 1), (1, -1), (2, 1), (2, -1), (3, 1), (3, -1)]):
        m = pshift[:, j * NP : (j + 1) * NP]
        for p in range(RVL):
            q = p - s if sign > 0 else p + s
            if 0 <= q < RVL:
                m[q, p] = 1.0
            else:
                m[127, p] = 1.0  # INF row
    pshift = pshift.astype(bf)

    in_maps = []
    for k in range(N_CORES):
        b, c = divmod(k, C)
        t16 = tgt[b].reshape(DD, PLANE).astype(np.int16)
        T = np.empty((128, PLANE), np.int16)
        T[0:48] = t16
        T[48:64] = 4  # sentinel: (4-c)^2 * 1024 >= 1024 isolates the blocks
        T[64:112] = t16
        T[112:128] = 4
        cvn = np.full((NP, 1), -32.0 * c, np.float32)
        cvc = np.full((NP, 1), float(c), np.float32)
        perm = [c] + [j for j in range(C) if j != c]
        # [h2*64+d, (c', h%24, w)] layout with class c in slot 0
        sm = (
            pred[b][perm]
            .reshape(C, DD, 2, 24, WW)
            .transpose(2, 1, 0, 3, 4)
            .reshape(2, 48, 4 * 1152)
        )
        in_maps.append(
            {
                "tgt": T,
                "cvn": cvn,
                "cvc": cvc,
                "ones": ones,
                "predsm": np.ascontiguousarray(sm),
                "pairmat": pairmat,
                "pshift": pshift,
            }
        )

    trace = bool(os.environ.get("BOUNDARY_KERNEL_TRACE"))
    if trace:
        import importlib.util

        if importlib.util.find_spec("antenv.axon_hooks") is None:
            trace = False
    res = run_bass_kernel_spmd(nc, in_maps, list(range(N_CORES)), trace=trace)
    global LAST_RESULTS
    LAST_RESULTS = res

    total = 0.0
    for k in range(N_CORES):
        b, c = divmod(k, C)
        if has_pos[(b, c)]:
            total += float(res.results[k]["out"].astype(np.float64).sum())
    return np.float32(total / (B * C * NVOX))


if __name__ == "__main__":
    import reference

    inputs = reference.setup_inputs()
    out = kernel(**{k: np.asarray(v) for k, v in inputs.items()})
    print("kernel out:", out)


# revision 12
# speedup vs baseline: 2.9484x; 1.0356x over previous
"""Trainium2 Bass kernel for nn_BoundaryLoss: mean(|softmax(pred) * SDF(onehot(target))|).

Strategy (8 NeuronCores, SPMD, one (b, c) pair per core):
  - Exact 3D squared EDT of the class-c mask (pos) and complement (neg) via
    separable passes. |sdf| = sqrt(g_pos) + sqrt(g_neg) since exactly one of
    the two is zero at every voxel.
  - W pass: two tensor_tensor_scan ops (fwd chamfer, then bwd chamfer chained
    on the fwd result) give the exact 1D L1 seed distance per row; one Square
    activation turns it into the squared EDT. INF pads between rows stop the
    recurrence from leaking across (h) rows.
  - H pass: 6 scalar_tensor_tensor min-plus shift updates (s = 1..3, both
    directions, truncation certified exact on the host). First op fused with
    the pass-init copy.
  - D pass: same, with the partition-axis shifts materialized by SBUF->SBUF
    DMA copies (borders filled from an INF tile; compute ops never straddle
    non-aligned partition starts).
  - Softmax: pred DMA'd in layout [h2*64+d, (c, h%24, w)] with class c
    permuted to slot 0; exp on the scalar engine; 4-way denominator adds,
    reciprocal_approx_fast and the weight multiply on DVE; two fused
    tensor_tensor_reduce ops against the PE pair-sum produce 48 partials.
  - Host sums the 8x48 partials, applies the has_pos gate and the mean factor.

Engine budget: DVE ~42us is the critical path; Act (init/exp/sqrt), pool
(neg-init, pad memsets), PE (pair-sum matmul) and the DMA queue all hide
under it.
"""

import os
import sys

import numpy as np

B, C, DD, HH, WW = 2, 4, 48, 48, 48
PLANE = HH * WW  # 2304
WPAD = WW + 1  # 49: w row + INF pad column
NFP = HH * WPAD  # 2352 padded free size
NVOX = DD * PLANE
S_MAX = 16
S_DEV = 3  # device kernel is built for shift radius 3; larger -> fallback
N_CORES = 8
RV = 112  # end of valid rows: pos [0:48) | gap [48:64) | neg [64:112)
INF_I = 30000.0  # scan/pad infinity (int16 domain)
INF_SEED = 1024.0  # neg-mask seed infinity (= 32^2, matches Square scaling)

_nc_cache = {}
LAST_RESULTS = None  # test harness introspection


def _ensure_paths():
    for p in ("/opt/trn_rl_repo",):
        if os.path.isdir(p) and p not in sys.path:
            sys.path.insert(0, p)


def _edt_sq_trunc_np(f0, S):
    """Truncated-shift separable squared EDT (numpy, int32). Mirrors the device
    algorithm; used for the shift-bound certification and the fallback path."""
    f = f0.astype(np.int32)
    for ax in (2, 1, 0):
        g = f.copy()
        for s in range(1, S + 1):
            s2 = s * s
            sl_out = [slice(None)] * 3
            sl_in = [slice(None)] * 3
            sl_out[ax] = slice(s, None)
            sl_in[ax] = slice(None, -s)
            np.minimum(g[tuple(sl_out)], f[tuple(sl_in)] + s2, out=g[tuple(sl_out)])
            sl_out[ax] = slice(None, -s)
            sl_in[ax] = slice(s, None)
            np.minimum(g[tuple(sl_out)], f[tuple(sl_in)] + s2, out=g[tuple(sl_out)])
        f = g
    return f


def _certified_shift_bound(masks):
    """Smallest S such that the S-truncated separable EDT is provably exact for
    every seed mask in `masks` (max truncated distance <= S certifies that no
    winning chain was cut off). The device kernel's exact-W variant is then
    also exact, since relaxing one pass can only move the result toward the
    true EDT."""
    for S in range(1, S_MAX + 1):
        worst = 0
        for m in masks:
            f0 = np.where(m, 0, 30000).astype(np.int16)
            g = _edt_sq_trunc_np(f0, S)
            worst = max(worst, int(np.ceil(np.sqrt(float(g.max())))))
        if worst <= S:
            return S
    return S_MAX + 1


def _reference_fallback(pred, target):
    """Exact numpy replica of the reference for pathological inputs the device
    path does not cover (wrong shapes, class filling a volume, S > S_DEV)."""
    INF = 1e9
    pred = np.asarray(pred, np.float32)
    target = np.asarray(target)
    b_, c_ = pred.shape[0], pred.shape[1]
    n = np.arange(pred.shape[-1])

    def minplus(f):
        d2 = ((n[:, None] - n[None, :]) ** 2).astype(np.float32)
        return (f[..., None, :] + d2).min(axis=-1)

    def edt(src):
        f = np.where(src, 0.0, INF).astype(np.float32)
        for ax in (-3, -2, -1):
            f = np.moveaxis(minplus(np.moveaxis(f, ax, -1)), -1, ax)
        return np.sqrt(f)

    e = np.exp(pred - pred.max(axis=1, keepdims=True))
    sm = e / e.sum(axis=1, keepdims=True)
    total = 0.0
    for b in range(b_):
        for c in range(c_):
            pos = target[b] == c
            if not pos.any():
                continue
            sdf = edt(pos) - edt(~pos)
            total += float(np.abs(sm[b, c] * sdf).sum(dtype=np.float64))
    return np.float32(total / pred.size)


def _build_nc():
    _ensure_paths()
    import concourse.tile as tile
    from concourse import bacc, mybir

    i16 = mybir.dt.int16
    f32 = mybir.dt.float32
    bf16 = mybir.dt.bfloat16
    ALU = mybir.AluOpType
    ACT = mybir.ActivationFunctionType

    NP = 128

    nc = bacc.Bacc("TRN2", target_bir_lowering=False, debug=False)

    tgt_d = nc.dram_tensor("tgt", [128, PLANE], i16, kind="ExternalInput")
    cvn_d = nc.dram_tensor("cvn", [NP, 1], f32, kind="ExternalInput")
    cvc_d = nc.dram_tensor("cvc", [NP, 1], f32, kind="ExternalInput")
    ones_d = nc.dram_tensor("ones", [NP, NFP], i16, kind="ExternalInput")
    sm_d = nc.dram_tensor("predsm", [2, 48, 4 * 1152], f32, kind="ExternalInput")
    pm_d = nc.dram_tensor("pairmat", [NP, 48], bf16, kind="ExternalInput")
    psh_d = nc.dram_tensor("pshift", [NP, 6 * NP], bf16, kind="ExternalInput")
    out_d = nc.dram_tensor("out", [48, 4], f32, kind="ExternalOutput")

    with tile.TileContext(nc) as tc:
        with (
            tc.tile_pool(name="main", bufs=1) as pool,
            tc.tile_pool(name="psh", bufs=2, space="PSUM") as psh_pool,
            tc.tile_pool(name="psq", bufs=2, space="PSUM") as psq_pool,
        ):
            T64 = pool.tile([NP, PLANE], i16, tag="T64")
            CVN = pool.tile([NP, 1], f32, tag="CVN")
            CVC = pool.tile([NP, 1], f32, tag="CVC")
            ONES = pool.tile([NP, NFP], i16, tag="ONES")
            SM = pool.tile([NP, 4 * 1152], f32, tag="SM")
            SME = pool.tile([NP, 4 * 1152], bf16, tag="SME")
            DNA = pool.tile([NP, 1152], bf16, tag="DNA")
            DNB = pool.tile([NP, 1152], bf16, tag="DNB")
            PM = pool.tile([NP, 48], bf16, tag="PM")
            PSH = pool.tile([NP, 6 * NP], bf16, tag="PSH")
            F = pool.tile([NP, NFP], i16, tag="F")
            WF = pool.tile([NP, NFP], i16, tag="WF")
            WB = pool.tile([NP, NFP], i16, tag="WB")
            A = pool.tile([NP, PLANE], bf16, tag="A")
            Bt = pool.tile([NP, PLANE], bf16, tag="B")
            SQ = pool.tile([NP, PLANE], bf16, tag="SQ")
            DN = pool.tile([NP, 1152], f32, tag="DN")
            RC = pool.tile([NP, 1152], f32, tag="RC")
            WT = pool.tile([NP, 1152], f32, tag="WT")
            SCR = pool.tile([48, 1152], f32, tag="SCR")
            SCR2 = pool.tile([48, 1152], f32, tag="SCR2")
            ACO = pool.tile([48, 4], f32, tag="ACO")


            DU = pool.tile([1, 1], f32, tag="DU")
            nc.vector.memset(DU[:], 4.0)
            nc.scalar.activation(DU[:], DU[:], ACT.Square)  # preload act table 0

            # ---- input DMAs -------------------------------------------------
            nc.sync.dma_start(T64[:], tgt_d[:])
            nc.sync.dma_start(CVN[:], cvn_d[:])
            nc.sync.dma_start(CVC[:], cvc_d[:])
            nc.sync.dma_start(ONES[:], ones_d[:])
            nc.sync.dma_start(SM[0:48, :], sm_d[0])
            nc.sync.dma_start(SM[64:RV, :], sm_d[1])
            nc.sync.dma_start(PM[:], pm_d[:])
            nc.sync.dma_start(PSH[:], psh_d[:])

            Fp = F[:].rearrange("p (h w) -> p h w", w=WPAD)

            # pool: pad columns of F to INF, SQ tail to 0, neg-mask init
            nc.gpsimd.memset(Fp[0:RV, :, WW : WW + 1], INF_I)
            nc.gpsimd.memset(SQ[96:NP, :], 0.0)  # sqrt later rewrites [96:RV)
            nc.gpsimd.memset(Bt[96:NP, :], 100.0)  # rows [96:RV) rewritten by H

            # Act: pos-mask init f = (32*(t-c))^2 over pos+gap rows (gap rows
            # carry the t=4 sentinel -> >= INF_SEED, isolating the blocks)
            nc.scalar.activation(
                Fp[0:64, :, 0:WW],
                T64[0:64, :].rearrange("p (h w) -> p h w", w=WW),
                ACT.Square, bias=CVN[0:64, :], scale=32.0,
            )
            nc.vector.tensor_scalar(
                out=Fp[64:RV, :, 0:WW],
                in0=T64[64:RV, :].rearrange("p (h w) -> p h w", w=WW),
                scalar1=CVC[64:RV, :], scalar2=INF_SEED,
                op0=ALU.is_equal, op1=ALU.mult,
            )

            # ---- W pass: fwd scan, bwd scan chained on fwd, then square -----
            nc.vector.tensor_tensor_scan(
                out=WF[0:RV, :], data0=ONES[0:RV, :], data1=F[0:RV, :],
                initial=INF_I, op0=ALU.add, op1=ALU.min,
            )
            nc.vector.tensor_tensor_scan(
                out=WB[0:RV, ::-1], data0=ONES[0:RV, ::-1], data1=WF[0:RV, ::-1],
                initial=INF_I, op0=ALU.add, op1=ALU.min,
            )
            nc.scalar.activation(SME[0:RV, :], SM[0:RV, :], ACT.Exp)
            A3 = A[:].rearrange("p (h w) -> p h w", w=WW)
            B3 = Bt[:].rearrange("p (h w) -> p h w", w=WW)
            nc.scalar.activation(
                A3[0:RV, :, :],
                WB[:].rearrange("p (h w) -> p h w", w=WPAD)[0:RV, :, 0:WW],
                ACT.Square,
            )

            # ---- H pass: A -> Bt, 6 truncated min-plus shifts ---------------
            nc.vector.scalar_tensor_tensor(
                out=B3[0:RV, 1:HH, :], in0=A3[0:RV, 0 : HH - 1, :], scalar=1.0,
                in1=A3[0:RV, 1:HH, :], op0=ALU.add, op1=ALU.min,
            )
            nc.vector.tensor_copy(B3[0:RV, 0:1, :], A3[0:RV, 0:1, :])
            nc.vector.scalar_tensor_tensor(
                out=B3[0:RV, 0 : HH - 1, :], in0=A3[0:RV, 1:HH, :], scalar=1.0,
                in1=B3[0:RV, 0 : HH - 1, :], op0=ALU.add, op1=ALU.min,
            )
            for s in (2, 3):
                s2 = float(s * s)
                nc.vector.scalar_tensor_tensor(
                    out=B3[0:RV, s:HH, :], in0=A3[0:RV, 0 : HH - s, :],
                    scalar=s2, in1=B3[0:RV, s:HH, :], op0=ALU.add, op1=ALU.min,
                )
                nc.vector.scalar_tensor_tensor(
                    out=B3[0:RV, 0 : HH - s, :], in0=A3[0:RV, s:HH, :],
                    scalar=s2, in1=B3[0:RV, 0 : HH - s, :],
                    op0=ALU.add, op1=ALU.min,
                )

            # ---- D pass: partition shifts via PE 0/1-matmuls into PSUM ------
            shifts = [(1, 1), (1, -1), (2, 1), (2, -1), (3, 1), (3, -1)]
            for j, (s, sign) in enumerate(shifts):
                stat = PSH[:, j * NP : (j + 1) * NP]
                for piece in range(3):
                    pj = psh_pool.tile([NP, 768], f32, tag="sh", name="pj")
                    h0 = piece * 768
                    c0 = 0
                    while c0 < 768:
                        nn = min(512, 768 - c0)
                        nc.tensor.matmul(
                            pj[:, c0 : c0 + nn], stat,
                            Bt[:, h0 + c0 : h0 + c0 + nn],
                            start=True, stop=True,
                        )
                        c0 += nn
                    src_prev = Bt if j == 0 else A
                    nc.vector.scalar_tensor_tensor(
                        out=A[0:RV, h0 : h0 + 768], in0=pj[0:RV, :],
                        scalar=float(s * s),
                        in1=src_prev[0:RV, h0 : h0 + 768],
                        op0=ALU.add, op1=ALU.min,
                    )

            # ---- |sdf| = sqrt(g_pos) + sqrt(g_neg) --------------------------
            nc.scalar.activation(SQ[0:RV, 0:1152], A[0:RV, 0:1152], ACT.Sqrt)
            nc.scalar.activation(SQ[0:RV, 1152:2304], A[0:RV, 1152:2304], ACT.Sqrt)

            # ---- softmax weight for class c (slot 0 after host permute) -----
            nc.vector.tensor_tensor(
                DNA[0:RV, :], SME[0:RV, 0:1152], SME[0:RV, 1152:2304], ALU.add
            )
            nc.vector.tensor_tensor(
                DNB[0:RV, :], SME[0:RV, 2304:3456], SME[0:RV, 3456:4608], ALU.add
            )
            nc.vector.tensor_tensor(
                DN[0:RV, :], DNA[0:RV, :], DNB[0:RV, :], ALU.add
            )
            nc.vector.reciprocal_approx_fast(out=RC[0:RV, :], in_=DN[0:RV, :])
            nc.vector.tensor_tensor(
                WT[0:RV, :], SME[0:RV, 0:1152], RC[0:RV, :], ALU.mult
            )

            # ---- partial[d] = sum |sdf| * w_c  (TT mults + Act accum) -------
            for q in range(4):
                half = q // 2
                wrows = WT[0:48, :] if half == 0 else WT[64:RV, :]
                w0 = (q % 2) * 576
                scr = SCR if q % 2 == 0 else SCR2
                ps = psq_pool.tile([48, 576], f32, tag="pp", name="pshalf")
                h0 = q * 576
                c0 = 0
                while c0 < 576:
                    nn = min(512, 576 - c0)
                    nc.tensor.matmul(
                        ps[:, c0 : c0 + nn], PM[:], SQ[:, h0 + c0 : h0 + c0 + nn],
                        start=True, stop=True,
                    )
                    c0 += nn
                nc.vector.tensor_tensor(
                    scr[:, 0:576], ps[:], wrows[:, w0 : w0 + 576], ALU.mult
                )
                nc.scalar.activation(
                    scr[:, 0:576], scr[:, 0:576], ACT.Copy,
                    accum_out=ACO[:, q : q + 1],
                )
            nc.sync.dma_start(out_d[:], ACO[:])

    nc.compile()
    return nc


def kernel(pred, target):
    pred = np.ascontiguousarray(np.asarray(pred), dtype=np.float32)
    target = np.asarray(target)

    if pred.shape != (B, C, DD, HH, WW) or target.shape != (B, DD, HH, WW):
        return _reference_fallback(pred, target)

    tgt = target.astype(np.int64)
    masks = []
    has_pos = {}
    for b in range(B):
        for c in range(C):
            m = tgt[b] == c
            has_pos[(b, c)] = bool(m.any())
            if has_pos[(b, c)]:
                masks.append(m)
                mn = ~m
                if mn.any():
                    masks.append(mn)
                else:
                    return _reference_fallback(pred, target)  # class fills volume

    if _certified_shift_bound(masks) > S_DEV:
        return _reference_fallback(pred, target)

    _ensure_paths()
    from concourse.bass_utils import run_bass_kernel_spmd

    if 0 not in _nc_cache:
        _nc_cache[0] = _build_nc()
    nc = _nc_cache[0]

    NP = 128

    ones = np.ones((NP, NFP), np.int16)
    ones[:, WW::WPAD] = int(INF_I)
    try:
        import ml_dtypes

        bf = ml_dtypes.bfloat16
    except ImportError:  # pragma: no cover
        bf = np.float32
    pairmat = np.zeros((NP, 48), np.float32)
    pairmat[np.arange(48), np.arange(48)] = 1.0
    pairmat[64 + np.arange(48), np.arange(48)] = 1.0
    pairmat = pairmat.astype(bf)
    RVL = RV
    pshift = np.zeros((NP, 6 * NP), np.float32)
    for j, (s, sign) in enumerate([(1, 1), (1, -1), (2, 1), (2, -1), (3, 1), (3, -1)]):
        m = pshift[:, j * NP : (j + 1) * NP]
        for p in range(RVL):
            q = p - s if sign > 0 else p + s
            if 0 <= q < RVL:
                m[q, p] = 1.0
            else:
                m[127, p] = 1.0  # INF row
    pshift = pshift.astype(bf)

    in_maps = []
    for k in range(N_CORES):
        b, c = divmod(k, C)
        t16 = tgt[b].reshape(DD, PLANE).astype(np.int16)
        T = np.empty((128, PLANE), np.int16)
        T[0:48] = t16
        T[48:64] = 4  # sentinel: (4-c)^2 * 1024 >= 1024 isolates the blocks
        T[64:112] = t16
        T[112:128] = 4
        cvn = np.full((NP, 1), -32.0 * c, np.float32)
        cvc = np.full((NP, 1), float(c), np.float32)
        perm = [c] + [j for j in range(C) if j != c]
        # [h2*64+d, (c', h%24, w)] layout with class c in slot 0
        sm = (
            pred[b][perm]
            .reshape(C, DD, 2, 24, WW)
            .transpose(2, 1, 0, 3, 4)
            .reshape(2, 48, 4 * 1152)
        )
        in_maps.append(
            {
                "tgt": T,
                "cvn": cvn,
                "cvc": cvc,
                "ones": ones,
                "predsm": np.ascontiguousarray(sm),
                "pairmat": pairmat,
                "pshift": pshift,
            }
        )

    trace = bool(os.environ.get("BOUNDARY_KERNEL_TRACE"))
    if trace:
        import importlib.util

        if importlib.util.find_spec("antenv.axon_hooks") is None:
            trace = False
    res = run_bass_kernel_spmd(nc, in_maps, list(range(N_CORES)), trace=trace)
    global LAST_RESULTS
    LAST_RESULTS = res

    total = 0.0
    for k in range(N_CORES):
        b, c = divmod(k, C)
        if has_pos[(b, c)]:
            total += float(res.results[k]["out"].astype(np.float64).sum())
    return np.float32(total / (B * C * NVOX))


if __name__ == "__main__":
    import reference

    inputs = reference.setup_inputs()
    out = kernel(**{k: np.asarray(v) for k, v in inputs.items()})
    print("kernel out:", out)


# revision 13
# speedup vs baseline: 2.9501x; 1.0006x over previous
"""Trainium2 Bass kernel for nn_BoundaryLoss: mean(|softmax(pred) * SDF(onehot(target))|).

Strategy (8 NeuronCores, SPMD, one (b, c) pair per core):
  - Exact 3D squared EDT of the class-c mask (pos) and complement (neg) via
    separable passes. |sdf| = sqrt(g_pos) + sqrt(g_neg) since exactly one of
    the two is zero at every voxel.
  - W pass: two tensor_tensor_scan ops (fwd chamfer, then bwd chamfer chained
    on the fwd result) give the exact 1D L1 seed distance per row; one Square
    activation turns it into the squared EDT. INF pads between rows stop the
    recurrence from leaking across (h) rows.
  - H pass: 6 scalar_tensor_tensor min-plus shift updates (s = 1..3, both
    directions, truncation certified exact on the host). First op fused with
    the pass-init copy.
  - D pass: same, with the partition-axis shifts materialized by SBUF->SBUF
    DMA copies (borders filled from an INF tile; compute ops never straddle
    non-aligned partition starts).
  - Softmax: pred DMA'd in layout [h2*64+d, (c, h%24, w)] with class c
    permuted to slot 0; exp on the scalar engine; 4-way denominator adds,
    reciprocal_approx_fast and the weight multiply on DVE; two fused
    tensor_tensor_reduce ops against the PE pair-sum produce 48 partials.
  - Host sums the 8x48 partials, applies the has_pos gate and the mean factor.

Engine budget: DVE ~42us is the critical path; Act (init/exp/sqrt), pool
(neg-init, pad memsets), PE (pair-sum matmul) and the DMA queue all hide
under it.
"""

import os
import sys

import numpy as np

B, C, DD, HH, WW = 2, 4, 48, 48, 48
PLANE = HH * WW  # 2304
WPAD = WW + 1  # 49: w row + INF pad column
NFP = HH * WPAD  # 2352 padded free size
NVOX = DD * PLANE
S_MAX = 16
S_DEV = 3  # device kernel is built for shift radius 3; larger -> fallback
N_CORES = 8
RV = 112  # end of valid rows: pos [0:48) | gap [48:64) | neg [64:112)
INF_I = 30000.0  # scan/pad infinity (int16 domain)
INF_SEED = 1024.0  # neg-mask seed infinity (= 32^2, matches Square scaling)

_nc_cache = {}
LAST_RESULTS = None  # test harness introspection


def _ensure_paths():
    for p in ("/opt/trn_rl_repo",):
        if os.path.isdir(p) and p not in sys.path:
            sys.path.insert(0, p)


def _edt_sq_trunc_np(f0, S):
    """Truncated-shift separable squared EDT (numpy, int32). Mirrors the device
    algorithm; used for the shift-bound certification and the fallback path."""
    f = f0.astype(np.int32)
    for ax in (2, 1, 0):
        g = f.copy()
        for s in range(1, S + 1):
            s2 = s * s
            sl_out = [slice(None)] * 3
            sl_in = [slice(None)] * 3
            sl_out[ax] = slice(s, None)
            sl_in[ax] = slice(None, -s)
            np.minimum(g[tuple(sl_out)], f[tuple(sl_in)] + s2, out=g[tuple(sl_out)])
            sl_out[ax] = slice(None, -s)
            sl_in[ax] = slice(s, None)
            np.minimum(g[tuple(sl_out)], f[tuple(sl_in)] + s2, out=g[tuple(sl_out)])
        f = g
    return f


def _certified_shift_bound(masks):
    """Smallest S such that the S-truncated separable EDT is provably exact for
    every seed mask in `masks` (max truncated distance <= S certifies that no
    winning chain was cut off). The device kernel's exact-W variant is then
    also exact, since relaxing one pass can only move the result toward the
    true EDT."""
    for S in range(1, S_MAX + 1):
        worst = 0
        for m in masks:
            f0 = np.where(m, 0, 30000).astype(np.int16)
            g = _edt_sq_trunc_np(f0, S)
            worst = max(worst, int(np.ceil(np.sqrt(float(g.max())))))
        if worst <= S:
            return S
    return S_MAX + 1


def _reference_fallback(pred, target):
    """Exact numpy replica of the reference for pathological inputs the device
    path does not cover (wrong shapes, class filling a volume, S > S_DEV)."""
    INF = 1e9
    pred = np.asarray(pred, np.float32)
    target = np.asarray(target)
    b_, c_ = pred.shape[0], pred.shape[1]
    n = np.arange(pred.shape[-1])

    def minplus(f):
        d2 = ((n[:, None] - n[None, :]) ** 2).astype(np.float32)
        return (f[..., None, :] + d2).min(axis=-1)

    def edt(src):
        f = np.where(src, 0.0, INF).astype(np.float32)
        for ax in (-3, -2, -1):
            f = np.moveaxis(minplus(np.moveaxis(f, ax, -1)), -1, ax)
        return np.sqrt(f)

    e = np.exp(pred - pred.max(axis=1, keepdims=True))
    sm = e / e.sum(axis=1, keepdims=True)
    total = 0.0
    for b in range(b_):
        for c in range(c_):
            pos = target[b] == c
            if not pos.any():
                continue
            sdf = edt(pos) - edt(~pos)
            total += float(np.abs(sm[b, c] * sdf).sum(dtype=np.float64))
    return np.float32(total / pred.size)


def _build_nc():
    _ensure_paths()
    import concourse.tile as tile
    from concourse import bacc, mybir

    i16 = mybir.dt.int16
    f32 = mybir.dt.float32
    bf16 = mybir.dt.bfloat16
    ALU = mybir.AluOpType
    ACT = mybir.ActivationFunctionType

    NP = 128

    nc = bacc.Bacc("TRN2", target_bir_lowering=False, debug=False)

    tgt_d = nc.dram_tensor("tgt", [128, PLANE], i16, kind="ExternalInput")
    cvc_d = nc.dram_tensor("cvc", [NP, 1], f32, kind="ExternalInput")
    ones_d = nc.dram_tensor("ones", [NP, NFP], i16, kind="ExternalInput")
    sm_d = nc.dram_tensor("predsm", [2, 48, 4 * 1152], f32, kind="ExternalInput")
    pm_d = nc.dram_tensor("pairmat", [NP, 48], bf16, kind="ExternalInput")
    psh_d = nc.dram_tensor("pshift", [NP, 6 * NP], bf16, kind="ExternalInput")
    out_d = nc.dram_tensor("out", [48, 4], f32, kind="ExternalOutput")

    with tile.TileContext(nc) as tc:
        with (
            tc.tile_pool(name="main", bufs=1) as pool,
            tc.tile_pool(name="psh", bufs=2, space="PSUM") as psh_pool,
            tc.tile_pool(name="psq", bufs=2, space="PSUM") as psq_pool,
        ):
            T64 = pool.tile([NP, PLANE], i16, tag="T64")
            CVC = pool.tile([NP, 1], f32, tag="CVC")
            ONES = pool.tile([NP, NFP], i16, tag="ONES")
            SM = pool.tile([NP, 4 * 1152], f32, tag="SM")
            SME = pool.tile([NP, 4 * 1152], bf16, tag="SME")
            DNA = pool.tile([NP, 1152], bf16, tag="DNA")
            DNB = pool.tile([NP, 1152], bf16, tag="DNB")
            PM = pool.tile([NP, 48], bf16, tag="PM")
            PSH = pool.tile([NP, 6 * NP], bf16, tag="PSH")
            F = pool.tile([NP, NFP], i16, tag="F")
            WF = pool.tile([NP, NFP], i16, tag="WF")
            WB = pool.tile([NP, NFP], i16, tag="WB")
            A = pool.tile([NP, PLANE], bf16, tag="A")
            Bt = pool.tile([NP, PLANE], bf16, tag="B")
            SQ = pool.tile([NP, PLANE], bf16, tag="SQ")
            DN = pool.tile([NP, 1152], f32, tag="DN")
            RC = pool.tile([NP, 1152], f32, tag="RC")
            WT = pool.tile([NP, 1152], f32, tag="WT")
            SCR = pool.tile([48, 1152], f32, tag="SCR")
            SCR2 = pool.tile([48, 1152], f32, tag="SCR2")
            ACO = pool.tile([48, 4], f32, tag="ACO")


            DU = pool.tile([1, 1], f32, tag="DU")
            nc.vector.memset(DU[:], 4.0)
            nc.scalar.activation(DU[:], DU[:], ACT.Square)  # preload act table 0

            # ---- input DMAs -------------------------------------------------
            nc.sync.dma_start(T64[:], tgt_d[:])
            nc.sync.dma_start(CVC[:], cvc_d[:])
            nc.sync.dma_start(ONES[:], ones_d[:])
            nc.sync.dma_start(SM[0:48, :], sm_d[0])
            nc.sync.dma_start(SM[64:RV, :], sm_d[1])
            nc.sync.dma_start(PM[:], pm_d[:])
            nc.sync.dma_start(PSH[:], psh_d[:])

            Fp = F[:].rearrange("p (h w) -> p h w", w=WPAD)

            # pool: pad columns of F to INF, SQ tail to 0, neg-mask init
            nc.gpsimd.memset(Fp[0:RV, :, WW : WW + 1], INF_I)
            nc.gpsimd.memset(SQ[96:NP, :], 0.0)  # sqrt later rewrites [96:RV)
            nc.gpsimd.memset(Bt[96:NP, :], 100.0)  # rows [96:RV) rewritten by H

            # Act: pos-mask init f = (32*(t-c))^2 over pos+gap rows (gap rows
            # carry the t=4 sentinel -> >= INF_SEED, isolating the blocks)
            nc.vector.tensor_scalar(
                out=Fp[0:64, :, 0:WW],
                in0=T64[0:64, :].rearrange("p (h w) -> p h w", w=WW),
                scalar1=CVC[0:64, :], scalar2=INF_SEED,
                op0=ALU.not_equal, op1=ALU.mult,
            )
            nc.vector.tensor_scalar(
                out=Fp[64:RV, :, 0:WW],
                in0=T64[64:RV, :].rearrange("p (h w) -> p h w", w=WW),
                scalar1=CVC[64:RV, :], scalar2=INF_SEED,
                op0=ALU.is_equal, op1=ALU.mult,
            )

            # ---- W pass: fwd scan, bwd scan chained on fwd, then square -----
            nc.vector.tensor_tensor_scan(
                out=WF[0:RV, :], data0=ONES[0:RV, :], data1=F[0:RV, :],
                initial=INF_I, op0=ALU.add, op1=ALU.min,
            )
            nc.vector.tensor_tensor_scan(
                out=WB[0:RV, ::-1], data0=ONES[0:RV, ::-1], data1=WF[0:RV, ::-1],
                initial=INF_I, op0=ALU.add, op1=ALU.min,
            )
            nc.scalar.activation(SME[0:RV, :], SM[0:RV, :], ACT.Exp)
            A3 = A[:].rearrange("p (h w) -> p h w", w=WW)
            B3 = Bt[:].rearrange("p (h w) -> p h w", w=WW)
            nc.scalar.activation(
                A3[0:RV, :, :],
                WB[:].rearrange("p (h w) -> p h w", w=WPAD)[0:RV, :, 0:WW],
                ACT.Square,
            )

            # ---- H pass: A -> Bt, 6 truncated min-plus shifts ---------------
            nc.vector.scalar_tensor_tensor(
                out=B3[0:RV, 1:HH, :], in0=A3[0:RV, 0 : HH - 1, :], scalar=1.0,
                in1=A3[0:RV, 1:HH, :], op0=ALU.add, op1=ALU.min,
            )
            nc.vector.tensor_copy(B3[0:RV, 0:1, :], A3[0:RV, 0:1, :])
            nc.vector.scalar_tensor_tensor(
                out=B3[0:RV, 0 : HH - 1, :], in0=A3[0:RV, 1:HH, :], scalar=1.0,
                in1=B3[0:RV, 0 : HH - 1, :], op0=ALU.add, op1=ALU.min,
            )
            for s in (2, 3):
                s2 = float(s * s)
                nc.vector.scalar_tensor_tensor(
                    out=B3[0:RV, s:HH, :], in0=A3[0:RV, 0 : HH - s, :],
                    scalar=s2, in1=B3[0:RV, s:HH, :], op0=ALU.add, op1=ALU.min,
                )
                nc.vector.scalar_tensor_tensor(
                    out=B3[0:RV, 0 : HH - s, :], in0=A3[0:RV, s:HH, :],
                    scalar=s2, in1=B3[0:RV, 0 : HH - s, :],
                    op0=ALU.add, op1=ALU.min,
                )

            # ---- D pass: partition shifts via PE 0/1-matmuls into PSUM ------
            shifts = [(1, 1), (1, -1), (2, 1), (2, -1), (3, 1), (3, -1)]
            for j, (s, sign) in enumerate(shifts):
                stat = PSH[:, j * NP : (j + 1) * NP]
                for piece in range(3):
                    pj = psh_pool.tile([NP, 768], f32, tag="sh", name="pj")
                    h0 = piece * 768
                    c0 = 0
                    while c0 < 768:
                        nn = min(512, 768 - c0)
                        nc.tensor.matmul(
                            pj[:, c0 : c0 + nn], stat,
                            Bt[:, h0 + c0 : h0 + c0 + nn],
                            start=True, stop=True,
                        )
                        c0 += nn
                    src_prev = Bt if j == 0 else A
                    nc.vector.scalar_tensor_tensor(
                        out=A[0:RV, h0 : h0 + 768], in0=pj[0:RV, :],
                        scalar=float(s * s),
                        in1=src_prev[0:RV, h0 : h0 + 768],
                        op0=ALU.add, op1=ALU.min,
                    )

            # ---- |sdf| = sqrt(g_pos) + sqrt(g_neg) --------------------------
            nc.scalar.activation(SQ[0:RV, 0:1152], A[0:RV, 0:1152], ACT.Sqrt)
            nc.scalar.activation(SQ[0:RV, 1152:2304], A[0:RV, 1152:2304], ACT.Sqrt)

            # ---- softmax weight for class c (slot 0 after host permute) -----
            nc.vector.tensor_tensor(
                DNA[0:RV, :], SME[0:RV, 0:1152], SME[0:RV, 1152:2304], ALU.add
            )
            nc.vector.tensor_tensor(
                DNB[0:RV, :], SME[0:RV, 2304:3456], SME[0:RV, 3456:4608], ALU.add
            )
            nc.vector.tensor_tensor(
                DN[0:RV, :], DNA[0:RV, :], DNB[0:RV, :], ALU.add
            )
            nc.vector.reciprocal_approx_fast(out=RC[0:RV, :], in_=DN[0:RV, :])
            nc.vector.tensor_tensor(
                WT[0:RV, :], SME[0:RV, 0:1152], RC[0:RV, :], ALU.mult
            )

            # ---- partial[d] = sum |sdf| * w_c  (TT mults + Act accum) -------
            for q in range(4):
                half = q // 2
                wrows = WT[0:48, :] if half == 0 else WT[64:RV, :]
                w0 = (q % 2) * 576
                scr = SCR if q % 2 == 0 else SCR2
                ps = psq_pool.tile([48, 576], f32, tag="pp", name="pshalf")
                h0 = q * 576
                c0 = 0
                while c0 < 576:
                    nn = min(512, 576 - c0)
                    nc.tensor.matmul(
                        ps[:, c0 : c0 + nn], PM[:], SQ[:, h0 + c0 : h0 + c0 + nn],
                        start=True, stop=True,
                    )
                    c0 += nn
                nc.vector.tensor_tensor(
                    scr[:, 0:576], ps[:], wrows[:, w0 : w0 + 576], ALU.mult
                )
                nc.scalar.activation(
                    scr[:, 0:576], scr[:, 0:576], ACT.Copy,
                    accum_out=ACO[:, q : q + 1],
                )
            nc.sync.dma_start(out_d[:], ACO[:])

    nc.compile()
    return nc


def kernel(pred, target):
    pred = np.ascontiguousarray(np.asarray(pred), dtype=np.float32)
    target = np.asarray(target)

    if pred.shape != (B, C, DD, HH, WW) or target.shape != (B, DD, HH, WW):
        return _reference_fallback(pred, target)

    tgt = target.astype(np.int64)
    masks = []
    has_pos = {}
    for b in range(B):
        for c in range(C):
            m = tgt[b] == c
            has_pos[(b, c)] = bool(m.any())
            if has_pos[(b, c)]:
                masks.append(m)
                mn = ~m
                if mn.any():
                    masks.append(mn)
                else:
                    return _reference_fallback(pred, target)  # class fills volume

    if _certified_shift_bound(masks) > S_DEV:
        return _reference_fallback(pred, target)

    _ensure_paths()
    from concourse.bass_utils import run_bass_kernel_spmd

    if 0 not in _nc_cache:
        _nc_cache[0] = _build_nc()
    nc = _nc_cache[0]

    NP = 128

    ones = np.ones((NP, NFP), np.int16)
    ones[:, WW::WPAD] = int(INF_I)
    try:
        import ml_dtypes

        bf = ml_dtypes.bfloat16
    except ImportError:  # pragma: no cover
        bf = np.float32
    pairmat = np.zeros((NP, 48), np.float32)
    pairmat[np.arange(48), np.arange(48)] = 1.0
    pairmat[64 + np.arange(48), np.arange(48)] = 1.0
    pairmat = pairmat.astype(bf)
    RVL = RV
    pshift = np.zeros((NP, 6 * NP), np.float32)
    for j, (s, sign) in enumerate([(1, 1), (1, -1), (2, 1), (2, -1), (3, 1), (3, -1)]):
        m = pshift[:, j * NP : (j + 1) * NP]
        for p in range(RVL):
            q = p - s if sign > 0 else p + s
            if 0 <= q < RVL:
                m[q, p] = 1.0
            else:
                m[127, p] = 1.0  # INF row
    pshift = pshift.astype(bf)

    in_maps = []
    for k in range(N_CORES):
        b, c = divmod(k, C)
        t16 = tgt[b].reshape(DD, PLANE).astype(np.int16)
        T = np.empty((128, PLANE), np.int16)
        T[0:48] = t16
        T[48:64] = 4  # sentinel: (4-c)^2 * 1024 >= 1024 isolates the blocks
        T[64:112] = t16
        T[112:128] = 4
        cvc = np.full((NP, 1), float(c), np.float32)
        perm = [c] + [j for j in range(C) if j != c]
        # [h2*64+d, (c', h%24, w)] layout with class c in slot 0
        sm = (
            pred[b][perm]
            .reshape(C, DD, 2, 24, WW)
            .transpose(2, 1, 0, 3, 4)
            .reshape(2, 48, 4 * 1152)
        )
        in_maps.append(
            {
                "tgt": T,
                "cvc": cvc,
                "ones": ones,
                "predsm": np.ascontiguousarray(sm),
                "pairmat": pairmat,
                "pshift": pshift,
            }
        )

    trace = bool(os.environ.get("BOUNDARY_KERNEL_TRACE"))
    if trace:
        import importlib.util

        if importlib.util.find_spec("antenv.axon_hooks") is None:
            trace = False
    res = run_bass_kernel_spmd(nc, in_maps, list(range(N_CORES)), trace=trace)
    global LAST_RESULTS
    LAST_RESULTS = res

    total = 0.0
    for k in range(N_CORES):
        b, c = divmod(k, C)
        if has_pos[(b, c)]:
            total += float(res.results[k]["out"].astype(np.float64).sum())
    return np.float32(total / (B * C * NVOX))


if __name__ == "__main__":
    import reference

    inputs = reference.setup_inputs()
    out = kernel(**{k: np.asarray(v) for k, v in inputs.items()})
    print("kernel out:", out)


# revision 14
# speedup vs baseline: 3.0276x; 1.0262x over previous
"""Trainium2 Bass kernel for nn_BoundaryLoss: mean(|softmax(pred) * SDF(onehot(target))|).

Strategy (8 NeuronCores, SPMD, one (b, c) pair per core):
  - Exact 3D squared EDT of the class-c mask (pos) and complement (neg) via
    separable passes. |sdf| = sqrt(g_pos) + sqrt(g_neg) since exactly one of
    the two is zero at every voxel.
  - W pass: two tensor_tensor_scan ops (fwd chamfer, then bwd chamfer chained
    on the fwd result) give the exact 1D L1 seed distance per row; one Square
    activation turns it into the squared EDT. INF pads between rows stop the
    recurrence from leaking across (h) rows.
  - H pass: 6 scalar_tensor_tensor min-plus shift updates (s = 1..3, both
    directions, truncation certified exact on the host). First op fused with
    the pass-init copy.
  - D pass: same, with the partition-axis shifts materialized by SBUF->SBUF
    DMA copies (borders filled from an INF tile; compute ops never straddle
    non-aligned partition starts).
  - Softmax: pred DMA'd in layout [h2*64+d, (c, h%24, w)] with class c
    permuted to slot 0; exp on the scalar engine; 4-way denominator adds,
    reciprocal_approx_fast and the weight multiply on DVE; two fused
    tensor_tensor_reduce ops against the PE pair-sum produce 48 partials.
  - Host sums the 8x48 partials, applies the has_pos gate and the mean factor.

Engine budget: DVE ~42us is the critical path; Act (init/exp/sqrt), pool
(neg-init, pad memsets), PE (pair-sum matmul) and the DMA queue all hide
under it.
"""

import os
import sys

import numpy as np

B, C, DD, HH, WW = 2, 4, 48, 48, 48
PLANE = HH * WW  # 2304
WPAD = WW + 1  # 49: w row + INF pad column
NFP = HH * WPAD  # 2352 padded free size
NVOX = DD * PLANE
S_MAX = 16
S_DEV = 3  # device kernel is built for shift radius 3; larger -> fallback
N_CORES = 8
RV = 112  # end of valid rows: pos [0:48) | gap [48:64) | neg [64:112)
INF_I = 30000.0  # scan/pad infinity (int16 domain)
INF_SEED = 1024.0  # neg-mask seed infinity (= 32^2, matches Square scaling)

_nc_cache = {}
LAST_RESULTS = None  # test harness introspection


def _ensure_paths():
    for p in ("/opt/trn_rl_repo",):
        if os.path.isdir(p) and p not in sys.path:
            sys.path.insert(0, p)


def _edt_sq_trunc_np(f0, S):
    """Truncated-shift separable squared EDT (numpy, int32). Mirrors the device
    algorithm; used for the shift-bound certification and the fallback path."""
    f = f0.astype(np.int32)
    for ax in (2, 1, 0):
        g = f.copy()
        for s in range(1, S + 1):
            s2 = s * s
            sl_out = [slice(None)] * 3
            sl_in = [slice(None)] * 3
            sl_out[ax] = slice(s, None)
            sl_in[ax] = slice(None, -s)
            np.minimum(g[tuple(sl_out)], f[tuple(sl_in)] + s2, out=g[tuple(sl_out)])
            sl_out[ax] = slice(None, -s)
            sl_in[ax] = slice(s, None)
            np.minimum(g[tuple(sl_out)], f[tuple(sl_in)] + s2, out=g[tuple(sl_out)])
        f = g
    return f


def _certified_shift_bound(masks):
    """Smallest S such that the S-truncated separable EDT is provably exact for
    every seed mask in `masks` (max truncated distance <= S certifies that no
    winning chain was cut off). The device kernel's exact-W variant is then
    also exact, since relaxing one pass can only move the result toward the
    true EDT."""
    for S in range(1, S_MAX + 1):
        worst = 0
        for m in masks:
            f0 = np.where(m, 0, 30000).astype(np.int16)
            g = _edt_sq_trunc_np(f0, S)
            worst = max(worst, int(np.ceil(np.sqrt(float(g.max())))))
        if worst <= S:
            return S
    return S_MAX + 1


def _reference_fallback(pred, target):
    """Exact numpy replica of the reference for pathological inputs the device
    path does not cover (wrong shapes, class filling a volume, S > S_DEV)."""
    INF = 1e9
    pred = np.asarray(pred, np.float32)
    target = np.asarray(target)
    b_, c_ = pred.shape[0], pred.shape[1]
    n = np.arange(pred.shape[-1])

    def minplus(f):
        d2 = ((n[:, None] - n[None, :]) ** 2).astype(np.float32)
        return (f[..., None, :] + d2).min(axis=-1)

    def edt(src):
        f = np.where(src, 0.0, INF).astype(np.float32)
        for ax in (-3, -2, -1):
            f = np.moveaxis(minplus(np.moveaxis(f, ax, -1)), -1, ax)
        return np.sqrt(f)

    e = np.exp(pred - pred.max(axis=1, keepdims=True))
    sm = e / e.sum(axis=1, keepdims=True)
    total = 0.0
    for b in range(b_):
        for c in range(c_):
            pos = target[b] == c
            if not pos.any():
                continue
            sdf = edt(pos) - edt(~pos)
            total += float(np.abs(sm[b, c] * sdf).sum(dtype=np.float64))
    return np.float32(total / pred.size)


def _build_nc():
    _ensure_paths()
    import concourse.tile as tile
    from concourse import bacc, mybir

    i16 = mybir.dt.int16
    f32 = mybir.dt.float32
    bf16 = mybir.dt.bfloat16
    ALU = mybir.AluOpType
    ACT = mybir.ActivationFunctionType

    NP = 128

    nc = bacc.Bacc("TRN2", target_bir_lowering=False, debug=False)

    tgt_d = nc.dram_tensor("tgt", [64, PLANE], i16, kind="ExternalInput")
    cvc_d = nc.dram_tensor("cvc", [NP, 1], f32, kind="ExternalInput")
    sm_d = nc.dram_tensor("predsm", [2, 48, 4 * 1152], f32, kind="ExternalInput")
    pm_d = nc.dram_tensor("pairmat", [NP, 48], bf16, kind="ExternalInput")
    psh_d = nc.dram_tensor("pshift", [NP, 6 * NP], bf16, kind="ExternalInput")
    out_d = nc.dram_tensor("out", [48, 4], f32, kind="ExternalOutput")

    with tile.TileContext(nc) as tc:
        with (
            tc.tile_pool(name="main", bufs=1) as pool,
            tc.tile_pool(name="psh", bufs=2, space="PSUM") as psh_pool,
            tc.tile_pool(name="psq", bufs=2, space="PSUM") as psq_pool,
        ):
            T64 = pool.tile([64, PLANE], i16, tag="T64")
            CVC = pool.tile([NP, 1], f32, tag="CVC")
            ONES = pool.tile([NP, NFP], i16, tag="ONES")
            SM = pool.tile([NP, 4 * 1152], f32, tag="SM")
            SME = pool.tile([NP, 4 * 1152], bf16, tag="SME")
            DNA = pool.tile([NP, 1152], bf16, tag="DNA")
            DNB = pool.tile([NP, 1152], bf16, tag="DNB")
            PM = pool.tile([NP, 48], bf16, tag="PM")
            PSH = pool.tile([NP, 6 * NP], bf16, tag="PSH")
            F = pool.tile([NP, NFP], i16, tag="F")
            WF = pool.tile([NP, NFP], i16, tag="WF")
            WB = pool.tile([NP, NFP], i16, tag="WB")
            A = pool.tile([NP, PLANE], bf16, tag="A")
            Bt = pool.tile([NP, PLANE], bf16, tag="B")
            SQ = pool.tile([NP, PLANE], bf16, tag="SQ")
            DN = pool.tile([NP, 1152], f32, tag="DN")
            RC = pool.tile([NP, 1152], f32, tag="RC")
            WT = pool.tile([NP, 1152], f32, tag="WT")
            SCR = pool.tile([48, 1152], f32, tag="SCR")
            SCR2 = pool.tile([48, 1152], f32, tag="SCR2")
            ACO = pool.tile([48, 4], f32, tag="ACO")


            DU = pool.tile([1, 1], f32, tag="DU")
            nc.vector.memset(DU[:], 4.0)
            nc.scalar.activation(DU[:], DU[:], ACT.Square)  # preload act table 0

            # ---- input DMAs -------------------------------------------------
            nc.sync.dma_start(T64[:], tgt_d[:])
            nc.sync.dma_start(CVC[:], cvc_d[:])
            nc.sync.dma_start(SM[0:48, :], sm_d[0])
            nc.sync.dma_start(SM[64:RV, :], sm_d[1])
            nc.sync.dma_start(PM[:], pm_d[:])
            nc.sync.dma_start(PSH[:], psh_d[:])

            Fp = F[:].rearrange("p (h w) -> p h w", w=WPAD)
            ONESp = ONES[:].rearrange("p (h w) -> p h w", w=WPAD)

            # pool: scan increment tile, F pad columns, tail zeros/INF
            nc.gpsimd.memset(ONES[0:RV, :], 1.0)
            nc.gpsimd.memset(ONESp[0:RV, :, WW : WW + 1], INF_I)
            nc.gpsimd.memset(Fp[0:RV, :, WW : WW + 1], INF_I)
            nc.gpsimd.memset(SQ[96:NP, :], 0.0)  # sqrt later rewrites [96:RV)
            nc.gpsimd.memset(Bt[96:NP, :], 100.0)  # rows [96:RV) rewritten by H

            # Act: pos-mask init f = (32*(t-c))^2 over pos+gap rows (gap rows
            # carry the t=4 sentinel -> >= INF_SEED, isolating the blocks)
            nc.vector.tensor_scalar(
                out=Fp[0:64, :, 0:WW],
                in0=T64[0:64, :].rearrange("p (h w) -> p h w", w=WW),
                scalar1=CVC[0:64, :], scalar2=INF_SEED,
                op0=ALU.not_equal, op1=ALU.mult,
            )
            nc.vector.tensor_scalar(
                out=Fp[64:RV, :, 0:WW],
                in0=T64[0:48, :].rearrange("p (h w) -> p h w", w=WW),
                scalar1=CVC[64:RV, :], scalar2=INF_SEED,
                op0=ALU.is_equal, op1=ALU.mult,
            )

            # ---- W pass: fwd scan, bwd scan chained on fwd, then square -----
            nc.vector.tensor_tensor_scan(
                out=WF[0:RV, :], data0=ONES[0:RV, :], data1=F[0:RV, :],
                initial=INF_I, op0=ALU.add, op1=ALU.min,
            )
            nc.vector.tensor_tensor_scan(
                out=WB[0:RV, ::-1], data0=ONES[0:RV, ::-1], data1=WF[0:RV, ::-1],
                initial=INF_I, op0=ALU.add, op1=ALU.min,
            )
            nc.scalar.activation(SME[0:RV, :], SM[0:RV, :], ACT.Exp)
            A3 = A[:].rearrange("p (h w) -> p h w", w=WW)
            B3 = Bt[:].rearrange("p (h w) -> p h w", w=WW)
            nc.scalar.activation(
                A3[0:RV, :, :],
                WB[:].rearrange("p (h w) -> p h w", w=WPAD)[0:RV, :, 0:WW],
                ACT.Square,
            )

            # ---- H pass: A -> Bt, 6 truncated min-plus shifts ---------------
            nc.vector.scalar_tensor_tensor(
                out=B3[0:RV, 1:HH, :], in0=A3[0:RV, 0 : HH - 1, :], scalar=1.0,
                in1=A3[0:RV, 1:HH, :], op0=ALU.add, op1=ALU.min,
            )
            nc.vector.tensor_copy(B3[0:RV, 0:1, :], A3[0:RV, 0:1, :])
            nc.vector.scalar_tensor_tensor(
                out=B3[0:RV, 0 : HH - 1, :], in0=A3[0:RV, 1:HH, :], scalar=1.0,
                in1=B3[0:RV, 0 : HH - 1, :], op0=ALU.add, op1=ALU.min,
            )
            for s in (2, 3):
                s2 = float(s * s)
                nc.vector.scalar_tensor_tensor(
                    out=B3[0:RV, s:HH, :], in0=A3[0:RV, 0 : HH - s, :],
                    scalar=s2, in1=B3[0:RV, s:HH, :], op0=ALU.add, op1=ALU.min,
                )
                nc.vector.scalar_tensor_tensor(
                    out=B3[0:RV, 0 : HH - s, :], in0=A3[0:RV, s:HH, :],
                    scalar=s2, in1=B3[0:RV, 0 : HH - s, :],
                    op0=ALU.add, op1=ALU.min,
                )

            # ---- D pass: partition shifts via PE 0/1-matmuls into PSUM ------
            shifts = [(1, 1), (1, -1), (2, 1), (2, -1), (3, 1), (3, -1)]
            for j, (s, sign) in enumerate(shifts):
                stat = PSH[:, j * NP : (j + 1) * NP]
                for piece in range(3):
                    pj = psh_pool.tile([NP, 768], f32, tag="sh", name="pj")
                    h0 = piece * 768
                    c0 = 0
                    while c0 < 768:
                        nn = min(512, 768 - c0)
                        nc.tensor.matmul(
                            pj[:, c0 : c0 + nn], stat,
                            Bt[:, h0 + c0 : h0 + c0 + nn],
                            start=True, stop=True,
                        )
                        c0 += nn
                    src_prev = Bt if j == 0 else A
                    nc.vector.scalar_tensor_tensor(
                        out=A[0:RV, h0 : h0 + 768], in0=pj[0:RV, :],
                        scalar=float(s * s),
                        in1=src_prev[0:RV, h0 : h0 + 768],
                        op0=ALU.add, op1=ALU.min,
                    )

            # ---- |sdf| = sqrt(g_pos) + sqrt(g_neg) --------------------------
            nc.scalar.activation(SQ[0:RV, 0:1152], A[0:RV, 0:1152], ACT.Sqrt)
            nc.scalar.activation(SQ[0:RV, 1152:2304], A[0:RV, 1152:2304], ACT.Sqrt)

            # ---- softmax weight for class c (slot 0 after host permute) -----
            nc.vector.tensor_tensor(
                DNA[0:RV, :], SME[0:RV, 0:1152], SME[0:RV, 1152:2304], ALU.add
            )
            nc.vector.tensor_tensor(
                DNB[0:RV, :], SME[0:RV, 2304:3456], SME[0:RV, 3456:4608], ALU.add
            )
            nc.vector.tensor_tensor(
                DN[0:RV, :], DNA[0:RV, :], DNB[0:RV, :], ALU.add
            )
            nc.vector.reciprocal_approx_fast(out=RC[0:RV, :], in_=DN[0:RV, :])
            nc.vector.tensor_tensor(
                WT[0:RV, :], SME[0:RV, 0:1152], RC[0:RV, :], ALU.mult
            )

            # ---- partial[d] = sum |sdf| * w_c  (TT mults + Act accum) -------
            for q in range(4):
                half = q // 2
                wrows = WT[0:48, :] if half == 0 else WT[64:RV, :]
                w0 = (q % 2) * 576
                scr = SCR if q % 2 == 0 else SCR2
                ps = psq_pool.tile([48, 576], f32, tag="pp", name="pshalf")
                h0 = q * 576
                c0 = 0
                while c0 < 576:
                    nn = min(512, 576 - c0)
                    nc.tensor.matmul(
                        ps[:, c0 : c0 + nn], PM[:], SQ[:, h0 + c0 : h0 + c0 + nn],
                        start=True, stop=True,
                    )
                    c0 += nn
                nc.vector.tensor_tensor(
                    scr[:, 0:576], ps[:], wrows[:, w0 : w0 + 576], ALU.mult
                )
                nc.scalar.activation(
                    scr[:, 0:576], scr[:, 0:576], ACT.Copy,
                    accum_out=ACO[:, q : q + 1],
                )
            nc.sync.dma_start(out_d[:], ACO[:])

    nc.compile()
    return nc


def kernel(pred, target):
    pred = np.ascontiguousarray(np.asarray(pred), dtype=np.float32)
    target = np.asarray(target)

    if pred.shape != (B, C, DD, HH, WW) or target.shape != (B, DD, HH, WW):
        return _reference_fallback(pred, target)

    tgt = target.astype(np.int64)
    masks = []
    has_pos = {}
    for b in range(B):
        for c in range(C):
            m = tgt[b] == c
            has_pos[(b, c)] = bool(m.any())
            if has_pos[(b, c)]:
                masks.append(m)
                mn = ~m
                if mn.any():
                    masks.append(mn)
                else:
                    return _reference_fallback(pred, target)  # class fills volume

    if _certified_shift_bound(masks) > S_DEV:
        return _reference_fallback(pred, target)

    _ensure_paths()
    from concourse.bass_utils import run_bass_kernel_spmd

    if 0 not in _nc_cache:
        _nc_cache[0] = _build_nc()
    nc = _nc_cache[0]

    NP = 128

    try:
        import ml_dtypes

        bf = ml_dtypes.bfloat16
    except ImportError:  # pragma: no cover
        bf = np.float32
    pairmat = np.zeros((NP, 48), np.float32)
    pairmat[np.arange(48), np.arange(48)] = 1.0
    pairmat[64 + np.arange(48), np.arange(48)] = 1.0
    pairmat = pairmat.astype(bf)
    RVL = RV
    pshift = np.zeros((NP, 6 * NP), np.float32)
    for j, (s, sign) in enumerate([(1, 1), (1, -1), (2, 1), (2, -1), (3, 1), (3, -1)]):
        m = pshift[:, j * NP : (j + 1) * NP]
        for p in range(RVL):
            q = p - s if sign > 0 else p + s
            if 0 <= q < RVL:
                m[q, p] = 1.0
            else:
                m[127, p] = 1.0  # INF row
    pshift = pshift.astype(bf)

    in_maps = []
    for k in range(N_CORES):
        b, c = divmod(k, C)
        t16 = tgt[b].reshape(DD, PLANE).astype(np.int16)
        T = np.empty((64, PLANE), np.int16)
        T[0:48] = t16
        T[48:64] = 4  # sentinel rows isolate pos block from neg block
        cvc = np.full((NP, 1), float(c), np.float32)
        perm = [c] + [j for j in range(C) if j != c]
        # [h2*64+d, (c', h%24, w)] layout with class c in slot 0
        sm = (
            pred[b][perm]
            .reshape(C, DD, 2, 24, WW)
            .transpose(2, 1, 0, 3, 4)
            .reshape(2, 48, 4 * 1152)
        )
        in_maps.append(
            {
                "tgt": T,
                "cvc": cvc,
                "predsm": np.ascontiguousarray(sm),
                "pairmat": pairmat,
                "pshift": pshift,
            }
        )

    trace = bool(os.environ.get("BOUNDARY_KERNEL_TRACE"))
    if trace:
        import importlib.util

        if importlib.util.find_spec("antenv.axon_hooks") is None:
            trace = False
    res = run_bass_kernel_spmd(nc, in_maps, list(range(N_CORES)), trace=trace)
    global LAST_RESULTS
    LAST_RESULTS = res

    total = 0.0
    for k in range(N_CORES):
        b, c = divmod(k, C)
        if has_pos[(b, c)]:
            total += float(res.results[k]["out"].astype(np.float64).sum())
    return np.float32(total / (B * C * NVOX))


if __name__ == "__main__":
    import reference

    inputs = reference.setup_inputs()
    out = kernel(**{k: np.asarray(v) for k, v in inputs.items()})
    print("kernel out:", out)
